# revision 47
# baseline (speedup 1.0000x reference)
"""Multi-head causal attention (B=4, T=2048, D=1024, H=16, HS=64) on 8 TRN2 cores.

Sharding: tensor-parallel over heads (2 heads/core) for QKV+attention, then an
AllToAll redistributes per-head context to token-parallel layout for the output
projection (each core projects 1024 tokens with the full Wp).

v2 structure (vs v1): the kernel is ACT(exp)/PE-paced, so everything else is
arranged to keep ScalarE doing exp only and TensorE dense:
  - all PSUM->SBUF evacuations run on DVE (ScalarE does exp exclusively)
  - softmax normalization is deferred past the AllToAll: phase B ships raw
    context rows plus per-(head,token) denominators (computed by K=128
    ones-column matmuls, col-packed with the two heads' AV matmuls); phase C
    does one fp32 reciprocal_approx_fast on DVE + one K=16 broadcast matmul
    per d-tile. No ACT reciprocal => no ACT table switching (the exp table
    set loads once). Context and denominators travel in separate AllToAlls
    (shards beyond 128KiB arrive corrupted; ctx is exactly 128KiB/shard).
  - emission interleaves phase A(b+1) matmuls between attention j-steps of
    batch b (and phase C0 between group-1 chunks) so the PE queue always has
    independent work behind score->exp->AV dependency stalls; scores are
    software-pipelined one j ahead of AV.
  - x-tile loads alternate between the sync and gpsimd DMA queues; a2a staging
    uses gpsimd; phase-C loads/stores use sync.

Compute dtype bf16 with fp32 PSUM accumulation everywhere.
"""
import os
import numpy as np

import concourse.bass as bass
import concourse.tile as tile
from concourse import bacc, mybir
from concourse.bass_utils import run_bass_kernel_spmd

f32 = mybir.dt.float32
f32r = mybir.dt.float32r
bf16 = mybir.dt.bfloat16

B, D, H, HS = 4, 1024, 16, 64
N_CORES = 8
HPC = H // N_CORES          # heads per core
QC = 512                    # q-chunk width
KT = 128                    # k-tile width
ND = D // 128               # din tiles

DT_NAME = "bf16"            # "f32r" | "bf16" | "f32"


def _np_dt(dt):
    import ml_dtypes
    return {f32: np.float32, f32r: np.float32, bf16: ml_dtypes.bfloat16}[dt]


def build_nc(T=2048, dt_name=DT_NAME):
    DT = {"f32r": f32r, "bf16": bf16, "f32": f32}[dt_name]
    BT = B * T
    SL = BT // N_CORES              # tokens per core in phase C
    NQC = T // QC                   # q-chunks per batch
    NTB = T // KT                   # k/t-tiles per batch
    NSPLIT = 2                      # a2a groups (g = qc % 2)
    HF = SL // NSPLIT
    assert HF == QC

    nc = bacc.Bacc("TRN2", target_bir_lowering=False, debug=False,
                   num_devices=N_CORES)

    xt_d = nc.dram_tensor("xt", [D, BT], DT, kind="ExternalInput").ap()
    wq_d = nc.dram_tensor("wq", [D, 128], DT, kind="ExternalInput").ap()
    wk_d = nc.dram_tensor("wk", [D, 128], DT, kind="ExternalInput").ap()
    wv_d = nc.dram_tensor("wv", [D, 128], DT, kind="ExternalInput").ap()
    wp_d = nc.dram_tensor("wp", [D, D], DT, kind="ExternalInput").ap()
    bp_d = nc.dram_tensor("bp", [D, 1], f32, kind="ExternalInput").ap()
    id_d = nc.dram_tensor("ident", [128, 128], DT, kind="ExternalInput").ap()
    tril_d = nc.dram_tensor("triu", [128, 128], DT, kind="ExternalInput").ap()
    onesc_d = nc.dram_tensor("onesc", [128, 4], DT, kind="ExternalInput").ap()
    sel_d = nc.dram_tensor("selT", [2 * ND, ND * 128], DT,
                           kind="ExternalInput").ap()
    out_d = nc.dram_tensor("outT", [D, SL], f32, kind="ExternalOutput").ap()

    DBG = bool(os.environ.get("KERN_DBG"))
    if DBG:
        dbg_q = nc.dram_tensor("dbg_q", [128, T], DT,
                               kind="ExternalOutput").ap()
        dbg_vA = nc.dram_tensor("dbg_vA", [128, (T // KT) * 128], DT,
                                kind="ExternalOutput").ap()
        dbg_avs = nc.dram_tensor("dbg_avs", [65, QC], DT,
                                 kind="ExternalOutput").ap()
        dbg_e = nc.dram_tensor("dbg_e", [128, 2 * QC], DT,
                               kind="ExternalOutput").ap()
        dbg_den = nc.dram_tensor("dbg_den", [2 * ND, HF], DT,
                                 kind="ExternalOutput").ap()
        dbg_rec = nc.dram_tensor("dbg_rec", [2 * ND, HF], DT,
                                 kind="ExternalOutput").ap()
        dbg_cxn = nc.dram_tensor("dbg_cxn", [128, HF], DT,
                                 kind="ExternalOutput").ap()
        dbg_dns = nc.dram_tensor("dbg_dns", [33, QC], DT,
                                 kind="ExternalOutput").ap()
        dbg_a2di = nc.dram_tensor("dbg_a2di", [N_CORES, 4, HF], DT,
                                  kind="ExternalOutput").ap()
        dbg_a2do = nc.dram_tensor("dbg_a2do", [N_CORES, 4, HF], DT,
                                  kind="ExternalOutput").ap()

    EXP = mybir.ActivationFunctionType.Exp

    with tile.TileContext(nc) as tc:
        with (
            tc.tile_pool(name="wts", bufs=1) as wts,
            tc.tile_pool(name="acts", bufs=1) as acts,
            tc.tile_pool(name="dram", bufs=1, space="DRAM") as dram,
        ):
            # ---- persistent loads ----
            wq_sb, wk_sb, wv_sb = [], [], []
            for j in range(ND):
                for lst, dd, nm in ((wq_sb, wq_d, "wq"), (wk_sb, wk_d, "wk"),
                                    (wv_sb, wv_d, "wv")):
                    t = wts.tile([128, 128], DT, name=f"{nm}{j}", tag=f"{nm}{j}")
                    nc.sync.dma_start(t[:], dd[j * 128:(j + 1) * 128, :])
                    lst.append(t)
            id_sb = wts.tile([128, 128], DT, name="id", tag="id")
            nc.sync.dma_start(id_sb[:], id_d[:])
            triu_sb = wts.tile([128, 128], DT, name="triu", tag="triu")
            nc.sync.dma_start(triu_sb[:], tril_d[:])
            onesc_sb = wts.tile([128, 4], DT, name="onesc", tag="onesc")
            nc.sync.dma_start(onesc_sb[:], onesc_d[:])
            sel_sb = wts.tile([2 * ND, ND * 128], DT, name="selT", tag="selT")
            nc.sync.dma_start(sel_sb[:], sel_d[:])

            # a2a buffers.  Shard payloads beyond 128KiB arrive corrupted
            # (measured: rows >=128 of a [8,130,512]bf16 and [8,132,512]bf16
            # buffer are garbage after the collective), so context rows
            # ([8,128,HF] = exactly 128KiB/shard, the known-good shape) and
            # denominator rows ([8,4,HF], one full 4KiB CCE chunk) travel in
            # separate AllToAlls.
            a2a_in = [dram.tile([N_CORES, 128, HF], DT, name=f"a2ai{g}")
                      for g in range(NSPLIT)]
            a2a_out = [dram.tile([N_CORES, 128, HF], DT, name=f"a2ao{g}")
                       for g in range(NSPLIT)]
            a2d_in = [dram.tile([N_CORES, 4, HF], DT, name=f"a2di{g}")
                      for g in range(NSPLIT)]
            a2d_out = [dram.tile([N_CORES, 4, HF], DT, name=f"a2do{g}")
                       for g in range(NSPLIT)]

            # per-batch activation tensors
            qT, kT, vA = [], [], []
            for b in range(B):
                qT.append(acts.tile([128, T], DT, name=f"qT{b}", tag=f"qT{b}"))
                kT.append(acts.tile([128, T], DT, name=f"kT{b}", tag=f"kT{b}"))
                vA.append(acts.tile([128, NTB * 128], DT, name=f"vA{b}",
                                    tag=f"vA{b}"))

            # wp/bp aren't needed until phase C — loaded late (just before
            # the first AllToAll) so phase A owns the DMA queues at startup
            wp_sb = []
            bp_sb = []

            def load_wp():
                for j in range(ND):
                    t = wts.tile([128, D], DT, name=f"wp{j}", tag=f"wp{j}")
                    nc.sync.dma_start(t[:], wp_d[j * 128:(j + 1) * 128, :])
                    wp_sb.append(t)
                for m in range(ND):
                    t = wts.tile([128, 1], f32, name=f"bp{m}", tag=f"bp{m}")
                    nc.sync.dma_start(t[:], bp_d[m * 128:(m + 1) * 128, :])
                    bp_sb.append(t)

            with (
                tc.tile_pool(name="pA", bufs=2) as pA,
                tc.tile_pool(name="pB", bufs=3) as pB,
                tc.tile_pool(name="pC", bufs=1) as pC,
            ):
                # PSUM budget (8 banks):
                #   A+g0 region: proj/tp ring 2 + sc 4 + av 2 = 8
                #   g1+C region: sc(+op) 4 + av 2 + bc 2 = 8
                # LIFO pool stack: psA (innermost) closes mid-kernel, psC
                # takes its banks
                psS_ctx = tc.tile_pool(name="psS", bufs=2, space="PSUM")
                psS = psS_ctx.__enter__()
                psAV_ctx = tc.tile_pool(name="psAV", bufs=1, space="PSUM")
                psAV = psAV_ctx.__enter__()
                psA_ctx = tc.tile_pool(name="psA", bufs=2, space="PSUM")
                psA = psA_ctx.__enter__()
                psC = None

                # ---------- phase A steps (one batch -> list of closures) ----
                def a_steps(b):
                    steps = []

                    for ch in range(NQC):
                        xt = []

                        def dma_x(ch=ch, xt=xt):
                            i0 = b * T + ch * QC
                            for j in range(ND):
                                t = pA.tile([128, QC], DT, name=f"x{j}",
                                            tag=f"x{j}", bufs=2)
                                eng = nc.sync if j % 2 == 0 else nc.gpsimd
                                eng.dma_start(
                                    t[:],
                                    xt_d[j * 128:(j + 1) * 128, i0:i0 + QC])
                                xt.append(t)
                        steps.append(dma_x)

                        # q and k projections: 2 half-steps each
                        for w_sb, dstl in ((wq_sb, qT), (wk_sb, kT)):
                            pp = []

                            def proj1(w_sb=w_sb, xt=xt, pp=pp):
                                t = psA.tile([128, QC], f32, name="pp",
                                             tag="proj", bufs=2)
                                pp.append(t)
                                for j in range(4):
                                    nc.tensor.matmul(t[:], w_sb[j][:],
                                                     xt[j][:],
                                                     start=(j == 0),
                                                     stop=False)

                            def proj2(w_sb=w_sb, dstl=dstl, ch=ch, xt=xt,
                                      pp=pp):
                                t = pp[0]
                                for j in range(4, ND):
                                    nc.tensor.matmul(t[:], w_sb[j][:],
                                                     xt[j][:], start=False,
                                                     stop=(j == ND - 1))
                                sl = slice(ch * QC, (ch + 1) * QC)
                                nc.vector.tensor_copy(dstl[b][:, sl], t[:])
                            steps.append(proj1)
                            steps.append(proj2)

                        # v projection + transposes
                        vst = []

                        def projv1(xt=xt, vst=vst):
                            t = psA.tile([128, QC], f32, name="pp",
                                         tag="proj", bufs=2)
                            vst.append(t)
                            for j in range(4):
                                nc.tensor.matmul(t[:], wv_sb[j][:], xt[j][:],
                                                 start=(j == 0), stop=False)

                        def projv2(xt=xt, vst=vst):
                            t = vst[0]
                            for j in range(4, ND):
                                nc.tensor.matmul(t[:], wv_sb[j][:], xt[j][:],
                                                 start=False,
                                                 stop=(j == ND - 1))
                            s = pA.tile([128, QC], DT, name="vst", tag="vst",
                                        bufs=2)
                            nc.vector.tensor_copy(s[:], t[:])
                            vst.append(s)
                        steps.append(projv1)
                        steps.append(projv2)

                        def trans(ch=ch, vst=vst, lo=0):
                            s = vst[1]
                            for blk in range(lo, lo + 2):
                                tp = psA.tile([128, QC], DT, name="tp",
                                              tag="proj", bufs=2)
                                nc.tensor.transpose(
                                    tp[:, 0:128],
                                    s[:, blk * 128:(blk + 1) * 128], id_sb[:])
                                slot = (ch * (QC // 128) + blk)
                                nc.vector.tensor_copy(
                                    vA[b][:, slot * 128:(slot + 1) * 128],
                                    tp[:, 0:128])
                        steps.append(lambda trans=trans: trans(lo=0))
                        steps.append(lambda trans=trans: trans(lo=2))
                    return steps

                # ---------- filler ----------
                class Filler:
                    def __init__(self):
                        self.q = []

                    def add(self, steps):
                        self.q.extend(steps)

                    def step(self, n=1):
                        for _ in range(n):
                            if self.q:
                                self.q.pop(0)()

                    def drain(self):
                        while self.q:
                            self.q.pop(0)()

                filler = Filler()

                # ---------- attention chunk ----------
                def attn_chunk(b, qc):
                    nj = 4 * qc + 4
                    # both heads' AV col-packed in one bank (h0 rows 0:64 at
                    # col-group 0, h1 rows 64:128 at col-group 64 — they run
                    # concurrently); denominators via K=128 ones-column
                    # matmuls into one bank (h0 at partition 0, h1 at 32)
                    av = psAV.tile([128, QC], f32, name="av", tag="av",
                                   bufs=1)
                    dn = psAV.tile([33, QC], f32, name="dn", tag="dn",
                                   bufs=1)
                    sc_t = {}
                    e_t = {}

                    def emit_scores(j):
                        jr = j - 4 * qc
                        off = max(jr, 0) * 128
                        w = QC - off
                        qsl = slice(qc * QC + off, (qc + 1) * QC)
                        sc = psS.tile([128, 2 * QC], f32, name="scb",
                                      tag="scb", bufs=2)
                        for h in range(HPC):
                            hp = slice(h * 64, (h + 1) * 64)
                            nc.tensor.matmul(
                                sc[:, h * QC:h * QC + w],
                                kT[b][hp, j * 128:(j + 1) * 128],
                                qT[b][hp, qsl], start=True, stop=True)
                        sc_t[j] = (sc, w)

                    def emit_exp_mask(j):
                        jr = j - 4 * qc
                        sc, w = sc_t.pop(j)
                        e = pB.tile([128, 2 * QC], DT, name="exb",
                                    tag="exb", bufs=3)
                        sc3 = sc[:].rearrange("p (two q) -> p two q",
                                              two=2)[:, :, 0:w]
                        e3 = e[:, 0:2 * w].rearrange("p (two q) -> p two q",
                                                     two=2)
                        nc.scalar.activation(e3, sc3, EXP,
                                             scale=1.0 / np.sqrt(HS))
                        if jr >= 0:
                            for h in range(HPC):
                                nc.vector.tensor_mul(
                                    e[:, h * w:h * w + 128],
                                    e[:, h * w:h * w + 128], triu_sb[:])
                        if DBG and b == 0 and qc == 0 and j == 0:
                            nc.sync.dma_start(dbg_e[:], e[:])
                        e_t[j] = (e, w)

                    def emit_av(j):
                        jr = j - 4 * qc
                        off = max(jr, 0) * 128
                        e, w = e_t.pop(j)
                        st, sp = (j == 0), (j == nj - 1)
                        for h in range(HPC):
                            lhs = vA[b][:, j * 128 + h * 64:
                                        j * 128 + (h + 1) * 64]
                            nc.tensor.matmul(av[h * 64:(h + 1) * 64, off:QC],
                                             lhs, e[:, h * w:(h + 1) * w],
                                             start=st, stop=sp)
                        for h in range(HPC):
                            nc.tensor.matmul(dn[32 * h:32 * h + 1, off:QC],
                                             onesc_sb[:, 2 * h:2 * h + 1],
                                             e[:, h * w:(h + 1) * w],
                                             start=st, stop=sp)

                    # software pipeline: scores one j ahead of AV
                    emit_scores(0)
                    for j in range(nj):
                        emit_exp_mask(j)
                        if j + 1 < nj:
                            emit_scores(j + 1)
                        # filler lands between S(j+1) and AV(j) in the PE
                        # queue, covering the wait for exp(j)
                        filler.step(2)
                        emit_av(j)

                    # drain: raw ctx + denominator rows to a2a staging
                    tok0 = b * T + qc * QC
                    d = tok0 // SL
                    g = qc % NSPLIT
                    avs = pB.tile([128, QC], DT, name="avs", tag="avs",
                                  bufs=6)
                    nc.vector.tensor_copy(avs[:], av[:])
                    dns = pB.tile([33, QC], DT, name="dns", tag="dns", bufs=6)
                    nc.vector.tensor_copy(dns[0:1, :], dn[0:1, :])
                    nc.vector.tensor_copy(dns[32:33, :], dn[32:33, :])
                    if DBG and b == 0 and qc == 0:
                        nc.sync.dma_start(dbg_avs[:], avs[0:65, :])
                        nc.sync.dma_start(dbg_dns[:], dns[:])
                    nc.gpsimd.dma_start(a2a_in[g][d], avs[:])
                    nc.gpsimd.dma_start(a2d_in[g][d, 0:1, :], dns[0:1, :])
                    nc.gpsimd.dma_start(a2d_in[g][d, 1:2, :], dns[32:33, :])

                # ---------- phase C (one half) -> list of closures ----------
                def c_steps(g, last=False):
                    steps = []
                    cx = []
                    cxn = [None] * ND
                    rec_b = []

                    def load():
                        eng = nc.sync
                        if DBG and g == 0:
                            eng.dma_start(dbg_a2do[:], a2d_out[0][:])
                        # den_b rows h-major: r = h*8 + d.  (A partition-dim
                        # split rearrange as DMA dst writes byte-shifted
                        # garbage for h>0 — extract with two plain DMAs.)
                        den_b = pC.tile([2 * ND, HF], DT, name=f"den{g}",
                                        tag="den", bufs=2)
                        eng.dma_start(den_b[0:ND, :], a2d_out[g][:, 0, :])
                        eng.dma_start(den_b[ND:2 * ND, :],
                                      a2d_out[g][:, 1, :])
                        den_f = pC.tile([2 * ND, HF], f32, name=f"denf{g}",
                                        tag="denf", bufs=2)
                        nc.vector.tensor_copy(den_f[:], den_b[:])
                        rec_f = pC.tile([2 * ND, HF], f32, name=f"recf{g}",
                                        tag="recf", bufs=2)
                        nc.vector.reciprocal_approx_fast(rec_f[:], den_f[:])
                        rb = pC.tile([2 * ND, HF], DT, name=f"rec{g}",
                                     tag="rec", bufs=2)
                        nc.vector.tensor_copy(rb[:], rec_f[:])
                        rec_b.append(rb)
                        if DBG and g == 0:
                            nc.sync.dma_start(dbg_den[:], den_b[:])
                            nc.sync.dma_start(dbg_rec[:], rb[:])
                        for j in range(ND):
                            t = pC.tile([128, HF], DT, name=f"cx{g}{j}",
                                        tag=f"cx{g}{j}", bufs=1)
                            eng.dma_start(t[:], a2a_out[g][j])
                            cx.append(t)
                    steps.append(load)

                    def norm(j0):
                        for j in range(j0, j0 + 2):
                            bc = psC.tile([128, HF], f32, name="bc",
                                          tag="bc", bufs=2)
                            nc.tensor.matmul(bc[:],
                                             sel_sb[:, j * 128:(j + 1) * 128],
                                             rec_b[0][:], start=True,
                                             stop=True)
                            t = pC.tile([128, HF], DT, name=f"cxn{g}{j}",
                                        tag=f"cxn{g}{j}", bufs=1)
                            nc.vector.tensor_mul(t[:], cx[j][:], bc[:])
                            if DBG and g == 0 and j == 0:
                                nc.sync.dma_start(dbg_cxn[:], t[:])
                            cxn[j] = t
                    for j0 in range(0, ND, 2):
                        steps.append(lambda j0=j0: norm(j0))

                    def proj_m(m):
                        op = psS.tile([128, QC], f32, name="op", tag="scb",
                                      bufs=2)
                        for j in range(ND):
                            nc.tensor.matmul(
                                op[:], wp_sb[j][:, m * 128:(m + 1) * 128],
                                cxn[j][:], start=(j == 0), stop=(j == ND - 1))
                        os_ = pC.tile([128, QC], f32, name="os", tag="os",
                                      bufs=2)
                        nc.vector.tensor_scalar_add(os_[:], op[:], bp_sb[m][:])
                        nc.sync.dma_start(
                            out_d[m * 128:(m + 1) * 128,
                                  g * HF:(g + 1) * HF], os_[:])
                    for m in range(ND):
                        steps.append(lambda m=m: proj_m(m))
                    return steps

                # ---------- emission schedule ----------
                GROUPS = ([0, 2], [1, 3])

                with nc.named_scope("phA0"):
                    for st in a_steps(0):
                        st()
                    if DBG:
                        nc.sync.dma_start(dbg_q[:], qT[0][:])
                        nc.sync.dma_start(dbg_vA[:], vA[0][:])

                with nc.named_scope("phB0"):
                    for b in range(B):
                        if b + 1 < B:
                            filler.add(a_steps(b + 1))
                        for qc in GROUPS[0]:
                            attn_chunk(b, qc)
                        filler.drain()

                load_wp()
                if DBG:
                    nc.sync.dma_start(dbg_a2di[:], a2d_in[0][:])
                nc.gpsimd.collective_compute(
                    "AllToAll", mybir.AluOpType.bypass,
                    replica_groups=[list(range(N_CORES))],
                    ins=[a2a_in[0].opt()], outs=[a2a_out[0].opt()])
                nc.gpsimd.collective_compute(
                    "AllToAll", mybir.AluOpType.bypass,
                    replica_groups=[list(range(N_CORES))],
                    ins=[a2d_in[0].opt()], outs=[a2d_out[0].opt()])

                # close psA (phase A done), open bc pool
                psA_ctx.__exit__(None, None, None)
                psC_ctx = tc.tile_pool(name="psC", bufs=2, space="PSUM")
                psC = psC_ctx.__enter__()

                with nc.named_scope("phB1"):
                    for b in range(B):
                        for qc in GROUPS[1]:
                            attn_chunk(b, qc)
                        if b == 0:
                            # delayed so the bc matmuls can't reach the PE
                            # queue head before the AllToAll completes
                            filler.add(c_steps(0))
                    with nc.named_scope("phC0"):
                        filler.drain()

                nc.gpsimd.collective_compute(
                    "AllToAll", mybir.AluOpType.bypass,
                    replica_groups=[list(range(N_CORES))],
                    ins=[a2d_in[1].opt()], outs=[a2d_out[1].opt()])
                nc.gpsimd.collective_compute(
                    "AllToAll", mybir.AluOpType.bypass,
                    replica_groups=[list(range(N_CORES))],
                    ins=[a2a_in[1].opt()], outs=[a2a_out[1].opt()])

                with nc.named_scope("phC1"):
                    for st in c_steps(1, last=True):
                        st()

                for c in (psC_ctx, psAV_ctx, psS_ctx):
                    c.__exit__(None, None, None)

    nc.compile()
    return nc


def prep_inputs(x, Wq, Wk, Wv, Wp, bp, T, dt_name=DT_NAME):
    """Host-side sharding/layout prep. Returns in_maps for the 8 cores."""
    DT = {"f32r": f32r, "bf16": bf16, "f32": f32}[dt_name]
    ndt = _np_dt(DT)
    BT = B * T
    NTB = T // KT

    x = np.asarray(x, np.float32)
    Wq = np.asarray(Wq, np.float32)
    Wk = np.asarray(Wk, np.float32)
    Wv = np.asarray(Wv, np.float32)
    Wp = np.asarray(Wp, np.float32)
    bp = np.asarray(bp, np.float32)

    xt = np.ascontiguousarray(x.reshape(BT, D).T).astype(ndt)
    wp = np.ascontiguousarray(Wp.T).astype(ndt)
    bpc = np.ascontiguousarray(bp.reshape(D, 1))
    ident = np.eye(128, dtype=np.float32).astype(ndt)
    triu = np.triu(np.ones((128, 128), np.float32)).astype(ndt)
    onesc = np.ones((128, 4), np.float32).astype(ndt)
    # selT[r, j*128 + p] = 1 iff r == 8*(p//64) + j  (recip rows are h-major:
    # r = h*8 + slot; cx_j partition p belongs to head h = p//64 of slot j)
    sel = np.zeros((2 * ND, ND * 128), np.float32)
    for j in range(ND):
        for p in range(128):
            sel[8 * (p // 64) + j, j * 128 + p] = 1.0
    sel = sel.astype(ndt)

    def wslice(W, c):
        # [H, D, HS] heads 2c,2c+1 -> [D, 128] as [d, (h_local, e)]
        return np.ascontiguousarray(
            W[2 * c:2 * c + 2].transpose(1, 0, 2).reshape(D, 2 * HS)).astype(ndt)

    in_maps = []
    for c in range(N_CORES):
        in_maps.append({
            "xt": xt, "wq": wslice(Wq, c), "wk": wslice(Wk, c),
            "wv": wslice(Wv, c), "wp": wp, "bp": bpc, "ident": ident,
            "triu": triu, "onesc": onesc, "selT": sel,
        })
    return in_maps


_NC_CACHE = {}


def kernel(x, Wq, Wk, Wv, Wp, bp):
    T = np.asarray(x).shape[1]
    key = (T, DT_NAME)
    if key not in _NC_CACHE:
        _NC_CACHE[key] = build_nc(T, DT_NAME)
    nc = _NC_CACHE[key]
    in_maps = prep_inputs(x, Wq, Wk, Wv, Wp, bp, T, DT_NAME)
    res = run_bass_kernel_spmd(nc, in_maps, list(range(N_CORES)))
    out = np.concatenate([res.results[c]["outT"].T for c in range(N_CORES)],
                         axis=0)
    return np.ascontiguousarray(out.reshape(B, T, D).astype(np.float32))


# revision 48
# speedup vs baseline: 1.1543x; 1.1543x over previous
"""Multi-head causal attention (B=4, T=2048, D=1024, H=16, HS=64) on 8 TRN2 cores.

Sharding: tensor-parallel over heads (2 heads/core) for QKV+attention, then an
AllToAll redistributes per-head context to token-parallel layout for the output
projection (each core projects 1024 tokens with the full Wp).

v2 structure (vs v1): the kernel is ACT(exp)/PE-paced, so everything else is
arranged to keep ScalarE doing exp only and TensorE dense:
  - all PSUM->SBUF evacuations run on DVE (ScalarE does exp exclusively)
  - softmax normalization is deferred past the AllToAll: phase B ships raw
    context rows plus per-(head,token) denominators (computed by K=128
    ones-column matmuls, col-packed with the two heads' AV matmuls); phase C
    does one fp32 reciprocal_approx_fast on DVE + one K=16 broadcast matmul
    per d-tile. No ACT reciprocal => no ACT table switching (the exp table
    set loads once). Context and denominators travel in separate AllToAlls
    (shards beyond 128KiB arrive corrupted; ctx is exactly 128KiB/shard).
  - emission interleaves phase A(b+1) matmuls between attention j-steps of
    batch b (and phase C0 between group-1 chunks) so the PE queue always has
    independent work behind score->exp->AV dependency stalls; scores are
    software-pipelined one j ahead of AV.
  - x-tile loads alternate between the sync and gpsimd DMA queues; a2a staging
    uses gpsimd; phase-C loads/stores use sync.

Compute dtype bf16 with fp32 PSUM accumulation everywhere.
"""
import os
import numpy as np

import concourse.bass as bass
import concourse.tile as tile
from concourse import bacc, mybir
from concourse.bass_utils import run_bass_kernel_spmd

f32 = mybir.dt.float32
f32r = mybir.dt.float32r
bf16 = mybir.dt.bfloat16

B, D, H, HS = 4, 1024, 16, 64
N_CORES = 8
HPC = H // N_CORES          # heads per core
QC = 512                    # q-chunk width
KT = 128                    # k-tile width
ND = D // 128               # din tiles

DT_NAME = "bf16"            # "f32r" | "bf16" | "f32"


def _np_dt(dt):
    import ml_dtypes
    return {f32: np.float32, f32r: np.float32, bf16: ml_dtypes.bfloat16}[dt]


def build_nc(T=2048, dt_name=DT_NAME):
    DT = {"f32r": f32r, "bf16": bf16, "f32": f32}[dt_name]
    BT = B * T
    SL = BT // N_CORES              # tokens per core in phase C
    NQC = T // QC                   # q-chunks per batch
    NTB = T // KT                   # k/t-tiles per batch
    NSPLIT = 2                      # a2a groups (g = qc % 2)
    HF = SL // NSPLIT
    assert HF == QC

    nc = bacc.Bacc("TRN2", target_bir_lowering=False, debug=False,
                   num_devices=N_CORES)

    xt_d = nc.dram_tensor("xt", [D, BT], DT, kind="ExternalInput").ap()
    wq_d = nc.dram_tensor("wq", [D, 128], DT, kind="ExternalInput").ap()
    wk_d = nc.dram_tensor("wk", [D, 128], DT, kind="ExternalInput").ap()
    wv_d = nc.dram_tensor("wv", [D, 128], DT, kind="ExternalInput").ap()
    wp_d = nc.dram_tensor("wp", [D, D], DT, kind="ExternalInput").ap()
    bp_d = nc.dram_tensor("bp", [D, 1], f32, kind="ExternalInput").ap()
    id_d = nc.dram_tensor("ident", [128, 128], DT, kind="ExternalInput").ap()
    tril_d = nc.dram_tensor("triu", [128, 128], DT, kind="ExternalInput").ap()
    onesc_d = nc.dram_tensor("onesc", [128, 4], DT, kind="ExternalInput").ap()
    sel_d = nc.dram_tensor("selT", [2 * ND, ND * 128], DT,
                           kind="ExternalInput").ap()
    out_d = nc.dram_tensor("outT", [D, SL], f32, kind="ExternalOutput").ap()

    DBG = bool(os.environ.get("KERN_DBG"))
    if DBG:
        dbg_q = nc.dram_tensor("dbg_q", [128, T], DT,
                               kind="ExternalOutput").ap()
        dbg_vA = nc.dram_tensor("dbg_vA", [128, (T // KT) * 128], DT,
                                kind="ExternalOutput").ap()
        dbg_avs = nc.dram_tensor("dbg_avs", [65, QC], DT,
                                 kind="ExternalOutput").ap()
        dbg_e = nc.dram_tensor("dbg_e", [128, 2 * QC], DT,
                               kind="ExternalOutput").ap()
        dbg_den = nc.dram_tensor("dbg_den", [2 * ND, HF], DT,
                                 kind="ExternalOutput").ap()
        dbg_rec = nc.dram_tensor("dbg_rec", [2 * ND, HF], DT,
                                 kind="ExternalOutput").ap()
        dbg_cxn = nc.dram_tensor("dbg_cxn", [128, HF], DT,
                                 kind="ExternalOutput").ap()
        dbg_dns = nc.dram_tensor("dbg_dns", [33, QC], DT,
                                 kind="ExternalOutput").ap()
        dbg_a2di = nc.dram_tensor("dbg_a2di", [N_CORES, 4, HF], DT,
                                  kind="ExternalOutput").ap()
        dbg_a2do = nc.dram_tensor("dbg_a2do", [N_CORES, 4, HF], DT,
                                  kind="ExternalOutput").ap()

    EXP = mybir.ActivationFunctionType.Exp

    with tile.TileContext(nc) as tc:
        with (
            tc.tile_pool(name="wts", bufs=1) as wts,
            tc.tile_pool(name="acts", bufs=1) as acts,
            tc.tile_pool(name="dram", bufs=1, space="DRAM") as dram,
        ):
            # ---- persistent loads ----
            wq_sb, wk_sb, wv_sb = [], [], []
            for j in range(ND):
                for lst, dd, nm in ((wq_sb, wq_d, "wq"), (wk_sb, wk_d, "wk"),
                                    (wv_sb, wv_d, "wv")):
                    t = wts.tile([128, 128], DT, name=f"{nm}{j}", tag=f"{nm}{j}")
                    nc.sync.dma_start(t[:], dd[j * 128:(j + 1) * 128, :])
                    lst.append(t)
            id_sb = wts.tile([128, 128], DT, name="id", tag="id")
            nc.sync.dma_start(id_sb[:], id_d[:])
            triu_sb = wts.tile([128, 128], DT, name="triu", tag="triu")
            nc.sync.dma_start(triu_sb[:], tril_d[:])
            onesc_sb = wts.tile([128, 4], DT, name="onesc", tag="onesc")
            nc.sync.dma_start(onesc_sb[:], onesc_d[:])
            sel_sb = wts.tile([2 * ND, ND * 128], DT, name="selT", tag="selT")
            nc.sync.dma_start(sel_sb[:], sel_d[:])

            # a2a buffers.  Shard payloads beyond 128KiB arrive corrupted
            # (measured: rows >=128 of a [8,130,512]bf16 and [8,132,512]bf16
            # buffer are garbage after the collective), so context rows
            # ([8,128,HF] = exactly 128KiB/shard, the known-good shape) and
            # denominator rows ([8,4,HF], one full 4KiB CCE chunk) travel in
            # separate AllToAlls.
            a2a_in = [dram.tile([N_CORES, 128, HF], DT, name=f"a2ai{g}")
                      for g in range(NSPLIT)]
            a2a_out = [dram.tile([N_CORES, 128, HF], DT, name=f"a2ao{g}")
                       for g in range(NSPLIT)]
            a2d_in = [dram.tile([N_CORES, 4, HF], DT, name=f"a2di{g}")
                      for g in range(NSPLIT)]
            a2d_out = [dram.tile([N_CORES, 4, HF], DT, name=f"a2do{g}")
                       for g in range(NSPLIT)]

            # per-batch activation tensors
            qT, kT, vA = [], [], []
            for b in range(B):
                qT.append(acts.tile([128, T], DT, name=f"qT{b}", tag=f"qT{b}"))
                kT.append(acts.tile([128, T], DT, name=f"kT{b}", tag=f"kT{b}"))
                vA.append(acts.tile([128, NTB * 128], DT, name=f"vA{b}",
                                    tag=f"vA{b}"))

            # wp/bp aren't needed until phase C — loaded late (just before
            # the first AllToAll) so phase A owns the DMA queues at startup
            wp_sb = []
            bp_sb = []

            def load_wp():
                for j in range(ND):
                    t = wts.tile([128, D], DT, name=f"wp{j}", tag=f"wp{j}")
                    nc.sync.dma_start(t[:], wp_d[j * 128:(j + 1) * 128, :])
                    wp_sb.append(t)
                for m in range(ND):
                    t = wts.tile([128, 1], f32, name=f"bp{m}", tag=f"bp{m}")
                    nc.sync.dma_start(t[:], bp_d[m * 128:(m + 1) * 128, :])
                    bp_sb.append(t)

            with (
                tc.tile_pool(name="pA", bufs=2) as pA,
                tc.tile_pool(name="pB", bufs=3) as pB,
                tc.tile_pool(name="pC", bufs=1) as pC,
            ):
                # PSUM budget (8 banks):
                #   A+g0 region: proj/tp ring 2 + sc 4 + av 2 = 8
                #   g1+C region: sc(+op) 4 + av 2 + bc 2 = 8
                # LIFO pool stack: psA (innermost) closes mid-kernel, psC
                # takes its banks
                psS_ctx = tc.tile_pool(name="psS", bufs=2, space="PSUM")
                psS = psS_ctx.__enter__()
                psAV_ctx = tc.tile_pool(name="psAV", bufs=1, space="PSUM")
                psAV = psAV_ctx.__enter__()
                psA_ctx = tc.tile_pool(name="psA", bufs=2, space="PSUM")
                psA = psA_ctx.__enter__()
                psC = None

                # ---------- phase A steps (one batch -> list of closures) ----
                def a_steps(b):
                    steps = []

                    for ch in range(NQC):
                        xt = []

                        def dma_x(ch=ch, xt=xt):
                            i0 = b * T + ch * QC
                            for j in range(ND):
                                t = pA.tile([128, QC], DT, name=f"x{j}",
                                            tag=f"x{j}", bufs=2)
                                eng = nc.sync if j % 2 == 0 else nc.gpsimd
                                eng.dma_start(
                                    t[:],
                                    xt_d[j * 128:(j + 1) * 128, i0:i0 + QC])
                                xt.append(t)
                        steps.append(dma_x)

                        # q and k projections: 2 half-steps each
                        for w_sb, dstl in ((wq_sb, qT), (wk_sb, kT)):
                            pp = []

                            def proj1(w_sb=w_sb, xt=xt, pp=pp):
                                t = psA.tile([128, QC], f32, name="pp",
                                             tag="proj", bufs=2)
                                pp.append(t)
                                for j in range(4):
                                    nc.tensor.matmul(t[:], w_sb[j][:],
                                                     xt[j][:],
                                                     start=(j == 0),
                                                     stop=False)

                            def proj2(w_sb=w_sb, dstl=dstl, ch=ch, xt=xt,
                                      pp=pp):
                                t = pp[0]
                                for j in range(4, ND):
                                    nc.tensor.matmul(t[:], w_sb[j][:],
                                                     xt[j][:], start=False,
                                                     stop=(j == ND - 1))
                                sl = slice(ch * QC, (ch + 1) * QC)
                                nc.vector.tensor_copy(dstl[b][:, sl], t[:])
                            steps.append(proj1)
                            steps.append(proj2)

                        # v projection + transposes
                        vst = []

                        def projv1(xt=xt, vst=vst):
                            t = psA.tile([128, QC], f32, name="pp",
                                         tag="proj", bufs=2)
                            vst.append(t)
                            for j in range(4):
                                nc.tensor.matmul(t[:], wv_sb[j][:], xt[j][:],
                                                 start=(j == 0), stop=False)

                        def projv2(xt=xt, vst=vst):
                            t = vst[0]
                            for j in range(4, ND):
                                nc.tensor.matmul(t[:], wv_sb[j][:], xt[j][:],
                                                 start=False,
                                                 stop=(j == ND - 1))
                            s = pA.tile([128, QC], DT, name="vst", tag="vst",
                                        bufs=2)
                            nc.vector.tensor_copy(s[:], t[:])
                            vst.append(s)
                        steps.append(projv1)
                        steps.append(projv2)

                        def trans(ch=ch, vst=vst, lo=0):
                            s = vst[1]
                            for blk in range(lo, lo + 2):
                                tp = psA.tile([128, QC], DT, name="tp",
                                              tag="proj", bufs=2)
                                nc.tensor.transpose(
                                    tp[:, 0:128],
                                    s[:, blk * 128:(blk + 1) * 128], id_sb[:])
                                slot = (ch * (QC // 128) + blk)
                                nc.vector.tensor_copy(
                                    vA[b][:, slot * 128:(slot + 1) * 128],
                                    tp[:, 0:128])
                        steps.append(lambda trans=trans: trans(lo=0))
                        steps.append(lambda trans=trans: trans(lo=2))
                    return steps

                # ---------- filler ----------
                class Filler:
                    def __init__(self):
                        self.q = []

                    def add(self, steps):
                        self.q.extend(steps)

                    def step(self, n=1):
                        for _ in range(n):
                            if self.q:
                                self.q.pop(0)()

                    def drain(self):
                        while self.q:
                            self.q.pop(0)()

                filler = Filler()

                # ---------- attention chunk ----------
                def attn_chunk(b, qc):
                    nj = 4 * qc + 4
                    # both heads' AV col-packed in one bank (h0 rows 0:64 at
                    # col-group 0, h1 rows 64:128 at col-group 64 — they run
                    # concurrently); denominators via K=128 ones-column
                    # matmuls into one bank (h0 at partition 0, h1 at 32)
                    av = psAV.tile([128, QC], f32, name="av", tag="av",
                                   bufs=1)
                    dn = psAV.tile([33, QC], f32, name="dn", tag="dn",
                                   bufs=1)
                    sc_t = {}
                    e_t = {}

                    def emit_scores(j):
                        jr = j - 4 * qc
                        off = max(jr, 0) * 128
                        w = QC - off
                        qsl = slice(qc * QC + off, (qc + 1) * QC)
                        sc = psS.tile([128, 2 * QC], f32, name="scb",
                                      tag="scb", bufs=2)
                        for h in range(HPC):
                            hp = slice(h * 64, (h + 1) * 64)
                            nc.tensor.matmul(
                                sc[:, h * QC:h * QC + w],
                                kT[b][hp, j * 128:(j + 1) * 128],
                                qT[b][hp, qsl], start=True, stop=True)
                        sc_t[j] = (sc, w)

                    def emit_exp_mask(j):
                        jr = j - 4 * qc
                        sc, w = sc_t.pop(j)
                        e = pB.tile([128, 2 * QC], DT, name="exb",
                                    tag="exb", bufs=3)
                        sc3 = sc[:].rearrange("p (two q) -> p two q",
                                              two=2)[:, :, 0:w]
                        e3 = e[:, 0:2 * w].rearrange("p (two q) -> p two q",
                                                     two=2)
                        nc.scalar.activation(e3, sc3, EXP,
                                             scale=1.0 / np.sqrt(HS))
                        if jr >= 0:
                            for h in range(HPC):
                                nc.vector.tensor_mul(
                                    e[:, h * w:h * w + 128],
                                    e[:, h * w:h * w + 128], triu_sb[:])
                        if DBG and b == 0 and qc == 0 and j == 0:
                            nc.sync.dma_start(dbg_e[:], e[:])
                        e_t[j] = (e, w)

                    def emit_av(j):
                        jr = j - 4 * qc
                        off = max(jr, 0) * 128
                        e, w = e_t.pop(j)
                        st, sp = (j == 0), (j == nj - 1)
                        for h in range(HPC):
                            lhs = vA[b][:, j * 128 + h * 64:
                                        j * 128 + (h + 1) * 64]
                            nc.tensor.matmul(av[h * 64:(h + 1) * 64, off:QC],
                                             lhs, e[:, h * w:(h + 1) * w],
                                             start=st, stop=sp)
                        for h in range(HPC):
                            nc.tensor.matmul(dn[32 * h:32 * h + 1, off:QC],
                                             onesc_sb[:, 2 * h:2 * h + 1],
                                             e[:, h * w:(h + 1) * w],
                                             start=st, stop=sp)

                    # software pipeline: scores one j ahead of AV
                    emit_scores(0)
                    for j in range(nj):
                        emit_exp_mask(j)
                        if j + 1 < nj:
                            emit_scores(j + 1)
                        # filler lands between S(j+1) and AV(j) in the PE
                        # queue, covering the wait for exp(j)
                        filler.step(2)
                        emit_av(j)

                    # drain: raw ctx + denominator rows to a2a staging
                    tok0 = b * T + qc * QC
                    d = tok0 // SL
                    g = qc % NSPLIT
                    avs = pB.tile([128, QC], DT, name="avs", tag="avs",
                                  bufs=6)
                    nc.vector.tensor_copy(avs[:], av[:])
                    dns = pB.tile([33, QC], DT, name="dns", tag="dns", bufs=6)
                    nc.vector.tensor_copy(dns[0:1, :], dn[0:1, :])
                    nc.vector.tensor_copy(dns[32:33, :], dn[32:33, :])
                    if DBG and b == 0 and qc == 0:
                        nc.sync.dma_start(dbg_avs[:], avs[0:65, :])
                        nc.sync.dma_start(dbg_dns[:], dns[:])
                    nc.gpsimd.dma_start(a2a_in[g][d], avs[:])
                    nc.gpsimd.dma_start(a2d_in[g][d, 0:1, :], dns[0:1, :])
                    nc.gpsimd.dma_start(a2d_in[g][d, 1:2, :], dns[32:33, :])

                # ---------- phase C (one half) -> list of closures ----------
                def c_steps(g, last=False):
                    steps = []
                    cx = []
                    cxn = [None] * ND
                    rec_b = []

                    def load():
                        eng = nc.sync
                        if DBG and g == 0:
                            eng.dma_start(dbg_a2do[:], a2d_out[0][:])
                        # den_b rows h-major: r = h*8 + d.  (A partition-dim
                        # split rearrange as DMA dst writes byte-shifted
                        # garbage for h>0 — extract with two plain DMAs.)
                        den_b = pC.tile([2 * ND, HF], DT, name=f"den{g}",
                                        tag="den", bufs=2)
                        eng.dma_start(den_b[0:ND, :], a2d_out[g][:, 0, :])
                        eng.dma_start(den_b[ND:2 * ND, :],
                                      a2d_out[g][:, 1, :])
                        den_f = pC.tile([2 * ND, HF], f32, name=f"denf{g}",
                                        tag="denf", bufs=2)
                        nc.vector.tensor_copy(den_f[:], den_b[:])
                        rec_f = pC.tile([2 * ND, HF], f32, name=f"recf{g}",
                                        tag="recf", bufs=2)
                        nc.vector.reciprocal_approx_fast(rec_f[:], den_f[:])
                        rb = pC.tile([2 * ND, HF], DT, name=f"rec{g}",
                                     tag="rec", bufs=2)
                        nc.vector.tensor_copy(rb[:], rec_f[:])
                        rec_b.append(rb)
                        if DBG and g == 0:
                            nc.sync.dma_start(dbg_den[:], den_b[:])
                            nc.sync.dma_start(dbg_rec[:], rb[:])
                        for j in range(ND):
                            t = pC.tile([128, HF], DT, name=f"cx{g}{j}",
                                        tag=f"cx{g}{j}", bufs=1)
                            eng.dma_start(t[:], a2a_out[g][j])
                            cx.append(t)
                    steps.append(load)

                    def norm(j0):
                        for j in range(j0, j0 + 2):
                            bc = psC.tile([128, HF], f32, name="bc",
                                          tag="bc", bufs=2)
                            nc.tensor.matmul(bc[:],
                                             sel_sb[:, j * 128:(j + 1) * 128],
                                             rec_b[0][:], start=True,
                                             stop=True)
                            t = pC.tile([128, HF], DT, name=f"cxn{g}{j}",
                                        tag=f"cxn{g}{j}", bufs=1)
                            nc.vector.tensor_mul(t[:], cx[j][:], bc[:])
                            if DBG and g == 0 and j == 0:
                                nc.sync.dma_start(dbg_cxn[:], t[:])
                            cxn[j] = t
                    for j0 in range(0, ND, 2):
                        steps.append(lambda j0=j0: norm(j0))

                    def proj_m(m):
                        op = psS.tile([128, QC], f32, name="op", tag="scb",
                                      bufs=2)
                        for j in range(ND):
                            nc.tensor.matmul(
                                op[:], wp_sb[j][:, m * 128:(m + 1) * 128],
                                cxn[j][:], start=(j == 0), stop=(j == ND - 1))
                        os_ = pC.tile([128, QC], f32, name="os", tag="os",
                                      bufs=2)
                        nc.vector.tensor_scalar_add(os_[:], op[:], bp_sb[m][:])
                        nc.sync.dma_start(
                            out_d[m * 128:(m + 1) * 128,
                                  g * HF:(g + 1) * HF], os_[:])
                    for m in range(ND):
                        steps.append(lambda m=m: proj_m(m))
                    return steps

                # ---------- emission schedule ----------
                GROUPS = ([0, 2], [1, 3])

                with nc.named_scope("phA0"):
                    for st in a_steps(0):
                        st()
                    if DBG:
                        nc.sync.dma_start(dbg_q[:], qT[0][:])
                        nc.sync.dma_start(dbg_vA[:], vA[0][:])

                with nc.named_scope("phB0"):
                    for b in range(B):
                        if b + 1 < B:
                            filler.add(a_steps(b + 1))
                        for qc in GROUPS[0]:
                            attn_chunk(b, qc)
                        filler.drain()

                # close psA / open psC BEFORE the collectives: the pool
                # transition emits a cross-engine sync, which placed after
                # the collectives stalls every engine until the AllToAll
                # completes (~45us dead zone + HAM re-throttle)
                psA_ctx.__exit__(None, None, None)
                psC_ctx = tc.tile_pool(name="psC", bufs=2, space="PSUM")
                psC = psC_ctx.__enter__()

                load_wp()
                if DBG:
                    nc.sync.dma_start(dbg_a2di[:], a2d_in[0][:])
                nc.gpsimd.collective_compute(
                    "AllToAll", mybir.AluOpType.bypass,
                    replica_groups=[list(range(N_CORES))],
                    ins=[a2a_in[0].opt()], outs=[a2a_out[0].opt()])
                nc.gpsimd.collective_compute(
                    "AllToAll", mybir.AluOpType.bypass,
                    replica_groups=[list(range(N_CORES))],
                    ins=[a2d_in[0].opt()], outs=[a2d_out[0].opt()])

                with nc.named_scope("phB1"):
                    for b in range(B):
                        for qc in GROUPS[1]:
                            attn_chunk(b, qc)

                # group-1 collectives BEFORE phC0: phC0's compute (which only
                # needs group 0's data) overlaps their transport
                nc.gpsimd.collective_compute(
                    "AllToAll", mybir.AluOpType.bypass,
                    replica_groups=[list(range(N_CORES))],
                    ins=[a2d_in[1].opt()], outs=[a2d_out[1].opt()])
                nc.gpsimd.collective_compute(
                    "AllToAll", mybir.AluOpType.bypass,
                    replica_groups=[list(range(N_CORES))],
                    ins=[a2a_in[1].opt()], outs=[a2a_out[1].opt()])

                with nc.named_scope("phC0"):
                    for st in c_steps(0):
                        st()
                with nc.named_scope("phC1"):
                    for st in c_steps(1, last=True):
                        st()

                for c in (psC_ctx, psAV_ctx, psS_ctx):
                    c.__exit__(None, None, None)

    nc.compile()
    return nc


def prep_inputs(x, Wq, Wk, Wv, Wp, bp, T, dt_name=DT_NAME):
    """Host-side sharding/layout prep. Returns in_maps for the 8 cores."""
    DT = {"f32r": f32r, "bf16": bf16, "f32": f32}[dt_name]
    ndt = _np_dt(DT)
    BT = B * T
    NTB = T // KT

    x = np.asarray(x, np.float32)
    Wq = np.asarray(Wq, np.float32)
    Wk = np.asarray(Wk, np.float32)
    Wv = np.asarray(Wv, np.float32)
    Wp = np.asarray(Wp, np.float32)
    bp = np.asarray(bp, np.float32)

    xt = np.ascontiguousarray(x.reshape(BT, D).T).astype(ndt)
    wp = np.ascontiguousarray(Wp.T).astype(ndt)
    bpc = np.ascontiguousarray(bp.reshape(D, 1))
    ident = np.eye(128, dtype=np.float32).astype(ndt)
    triu = np.triu(np.ones((128, 128), np.float32)).astype(ndt)
    onesc = np.ones((128, 4), np.float32).astype(ndt)
    # selT[r, j*128 + p] = 1 iff r == 8*(p//64) + j  (recip rows are h-major:
    # r = h*8 + slot; cx_j partition p belongs to head h = p//64 of slot j)
    sel = np.zeros((2 * ND, ND * 128), np.float32)
    for j in range(ND):
        for p in range(128):
            sel[8 * (p // 64) + j, j * 128 + p] = 1.0
    sel = sel.astype(ndt)

    def wslice(W, c):
        # [H, D, HS] heads 2c,2c+1 -> [D, 128] as [d, (h_local, e)]
        return np.ascontiguousarray(
            W[2 * c:2 * c + 2].transpose(1, 0, 2).reshape(D, 2 * HS)).astype(ndt)

    in_maps = []
    for c in range(N_CORES):
        in_maps.append({
            "xt": xt, "wq": wslice(Wq, c), "wk": wslice(Wk, c),
            "wv": wslice(Wv, c), "wp": wp, "bp": bpc, "ident": ident,
            "triu": triu, "onesc": onesc, "selT": sel,
        })
    return in_maps


_NC_CACHE = {}


def kernel(x, Wq, Wk, Wv, Wp, bp):
    T = np.asarray(x).shape[1]
    key = (T, DT_NAME)
    if key not in _NC_CACHE:
        _NC_CACHE[key] = build_nc(T, DT_NAME)
    nc = _NC_CACHE[key]
    in_maps = prep_inputs(x, Wq, Wk, Wv, Wp, bp, T, DT_NAME)
    res = run_bass_kernel_spmd(nc, in_maps, list(range(N_CORES)))
    out = np.concatenate([res.results[c]["outT"].T for c in range(N_CORES)],
                         axis=0)
    return np.ascontiguousarray(out.reshape(B, T, D).astype(np.float32))


# revision 62
# speedup vs baseline: 1.1990x; 1.0387x over previous
"""Multi-head causal attention (B=4, T=2048, D=1024, H=16, HS=64) on 8 TRN2 cores.

Sharding: tensor-parallel over heads (2 heads/core) for QKV+attention, then an
AllToAll redistributes per-head context to token-parallel layout for the output
projection (each core projects 1024 tokens with the full Wp).

v2 structure (vs v1): the kernel is ACT(exp)/PE-paced, so everything else is
arranged to keep ScalarE doing exp only and TensorE dense:
  - all PSUM->SBUF evacuations run on DVE (ScalarE does exp exclusively)
  - softmax normalization is deferred past the AllToAll: phase B ships raw
    context rows plus per-(head,token) denominators (computed by K=128
    ones-column matmuls, col-packed with the two heads' AV matmuls); phase C
    does one fp32 reciprocal_approx_fast on DVE + one K=16 broadcast matmul
    per d-tile. No ACT reciprocal => no ACT table switching (the exp table
    set loads once). Context and denominators travel in separate AllToAlls
    (shards beyond 128KiB arrive corrupted; ctx is exactly 128KiB/shard).
  - emission interleaves phase A(b+1) matmuls between attention j-steps of
    batch b (and phase C0 between group-1 chunks) so the PE queue always has
    independent work behind score->exp->AV dependency stalls; scores are
    software-pipelined one j ahead of AV.
  - x-tile loads alternate between the sync and gpsimd DMA queues; a2a staging
    uses gpsimd; phase-C loads/stores use sync.

Compute dtype bf16 with fp32 PSUM accumulation everywhere.
"""
import os
import numpy as np

import concourse.bass as bass
import concourse.tile as tile
from concourse import bacc, mybir
from concourse.bass_utils import run_bass_kernel_spmd

f32 = mybir.dt.float32
f32r = mybir.dt.float32r
bf16 = mybir.dt.bfloat16

B, D, H, HS = 4, 1024, 16, 64
N_CORES = 8
HPC = H // N_CORES          # heads per core
QC = 512                    # q-chunk width
KT = 128                    # k-tile width
ND = D // 128               # din tiles

DT_NAME = "bf16"            # "f32r" | "bf16" | "f32"


def _np_dt(dt):
    import ml_dtypes
    return {f32: np.float32, f32r: np.float32, bf16: ml_dtypes.bfloat16}[dt]


def build_nc(T=2048, dt_name=DT_NAME):
    DT = {"f32r": f32r, "bf16": bf16, "f32": f32}[dt_name]
    BT = B * T
    SL = BT // N_CORES              # tokens per core in phase C
    NQC = T // QC                   # q-chunks per batch
    NTB = T // KT                   # k/t-tiles per batch
    NSPLIT = 2                      # a2a groups (g = qc % 2)
    HF = SL // NSPLIT
    assert HF == QC

    nc = bacc.Bacc("TRN2", target_bir_lowering=False, debug=False,
                   num_devices=N_CORES)

    xt_d = nc.dram_tensor("xt", [D, BT], DT, kind="ExternalInput").ap()
    wq_d = nc.dram_tensor("wq", [D, 128], DT, kind="ExternalInput").ap()
    wk_d = nc.dram_tensor("wk", [D, 128], DT, kind="ExternalInput").ap()
    wv_d = nc.dram_tensor("wv", [D, 128], DT, kind="ExternalInput").ap()
    wp_d = nc.dram_tensor("wp", [D, D], DT, kind="ExternalInput").ap()
    bp_d = nc.dram_tensor("bp", [D, 1], f32, kind="ExternalInput").ap()
    id_d = nc.dram_tensor("ident", [128, 128], DT, kind="ExternalInput").ap()
    tril_d = nc.dram_tensor("triu", [128, 128], DT, kind="ExternalInput").ap()
    onesm_d = nc.dram_tensor("onesm", [128, NTB], DT, kind="ExternalInput").ap()
    sel_d = nc.dram_tensor("selT", [2 * ND, ND * 128], DT,
                           kind="ExternalInput").ap()
    out_d = nc.dram_tensor("outT", [D, SL], f32, kind="ExternalOutput").ap()

    DBG = bool(os.environ.get("KERN_DBG"))
    if DBG:
        dbg_q = nc.dram_tensor("dbg_q", [128, T], DT,
                               kind="ExternalOutput").ap()
        dbg_vA = nc.dram_tensor("dbg_vA", [128, (T // KT) * 128], DT,
                                kind="ExternalOutput").ap()
        dbg_avs = nc.dram_tensor("dbg_avs", [65, QC], DT,
                                 kind="ExternalOutput").ap()
        dbg_e = nc.dram_tensor("dbg_e", [128, 2 * QC], DT,
                               kind="ExternalOutput").ap()
        dbg_den = nc.dram_tensor("dbg_den", [2 * ND, HF], DT,
                                 kind="ExternalOutput").ap()
        dbg_rec = nc.dram_tensor("dbg_rec", [2 * ND, HF], DT,
                                 kind="ExternalOutput").ap()
        dbg_cxn = nc.dram_tensor("dbg_cxn", [128, HF], DT,
                                 kind="ExternalOutput").ap()
        dbg_dns = nc.dram_tensor("dbg_dns", [33, QC], DT,
                                 kind="ExternalOutput").ap()
        dbg_a2di = nc.dram_tensor("dbg_a2di", [N_CORES, 4, HF], DT,
                                  kind="ExternalOutput").ap()
        dbg_a2do = nc.dram_tensor("dbg_a2do", [N_CORES, 4, HF], DT,
                                  kind="ExternalOutput").ap()

    EXP = mybir.ActivationFunctionType.Exp

    with tile.TileContext(nc) as tc:
        with (
            tc.tile_pool(name="wts", bufs=1) as wts,
            tc.tile_pool(name="acts", bufs=1) as acts,
            tc.tile_pool(name="dram", bufs=1, space="DRAM") as dram,
        ):
            # ---- persistent tiles.  Each DMA costs ~600ns of queue issue
            # time, so only Wq (needed by the very first matmul) loads before
            # the first x chunk; the rest loads right after (load_rest) ----
            wq_sb, wk_sb, wv_sb = [], [], []
            for j in range(ND):
                for lst, nm in ((wq_sb, "wq"), (wk_sb, "wk"), (wv_sb, "wv")):
                    lst.append(wts.tile([128, 128], DT, name=f"{nm}{j}",
                                        tag=f"{nm}{j}"))
            id_sb = wts.tile([128, 128], DT, name="id", tag="id")
            onesm_sb = wts.tile([128, NTB], DT, name="onesm", tag="onesm")
            triu_sb = wts.tile([128, 128], DT, name="triu", tag="triu")
            sel_sb = wts.tile([2 * ND, ND * 128], DT, name="selT", tag="selT")

            for j in range(ND):
                eng = nc.sync if j % 2 == 0 else nc.gpsimd
                eng.dma_start(wq_sb[j][:], wq_d[j * 128:(j + 1) * 128, :])

            def load_rest():
                for lst, dd in ((wk_sb, wk_d), (wv_sb, wv_d)):
                    for j in range(ND):
                        eng = nc.sync if j % 2 == 0 else nc.gpsimd
                        eng.dma_start(lst[j][:],
                                      dd[j * 128:(j + 1) * 128, :])
                nc.gpsimd.dma_start(id_sb[:], id_d[:])
                nc.gpsimd.dma_start(onesm_sb[:], onesm_d[:])
                nc.gpsimd.dma_start(triu_sb[:], tril_d[:])
                nc.gpsimd.dma_start(sel_sb[:], sel_d[:])

            # a2a buffers.  Shard payloads beyond 128KiB arrive corrupted
            # (measured: rows >=128 of a [8,130,512]bf16 and [8,132,512]bf16
            # buffer are garbage after the collective), so context rows
            # ([8,128,HF] = exactly 128KiB/shard, the known-good shape) and
            # denominator rows ([8,4,HF], one full 4KiB CCE chunk) travel in
            # separate AllToAlls.
            a2a_in = [dram.tile([N_CORES, 128, HF], DT, name=f"a2ai{g}")
                      for g in range(NSPLIT)]
            a2a_out = [dram.tile([N_CORES, 128, HF], DT, name=f"a2ao{g}")
                       for g in range(NSPLIT)]
            a2d_in = [dram.tile([N_CORES, 4, HF], DT, name=f"a2di{g}")
                      for g in range(NSPLIT)]
            a2d_out = [dram.tile([N_CORES, 4, HF], DT, name=f"a2do{g}")
                       for g in range(NSPLIT)]

            # per-batch activation tensors
            qT, kT, vA = [], [], []
            for b in range(B):
                qT.append(acts.tile([128, T], DT, name=f"qT{b}", tag=f"qT{b}"))
                kT.append(acts.tile([128, T], DT, name=f"kT{b}", tag=f"kT{b}"))
                vA.append(acts.tile([128, NTB * 130], DT, name=f"vA{b}",
                                    tag=f"vA{b}"))

            # wp/bp aren't needed until phase C — loaded late (just before
            # the first AllToAll) so phase A owns the DMA queues at startup
            wp_sb = []
            bp_sb = []

            def load_wp():
                for j in range(ND):
                    t = wts.tile([128, D], DT, name=f"wp{j}", tag=f"wp{j}")
                    nc.sync.dma_start(t[:], wp_d[j * 128:(j + 1) * 128, :])
                    wp_sb.append(t)
                for m in range(ND):
                    t = wts.tile([128, 1], f32, name=f"bp{m}", tag=f"bp{m}")
                    nc.sync.dma_start(t[:], bp_d[m * 128:(m + 1) * 128, :])
                    bp_sb.append(t)

            with (
                tc.tile_pool(name="pA", bufs=2) as pA,
                tc.tile_pool(name="pB", bufs=3) as pB,
                tc.tile_pool(name="pC", bufs=1) as pC,
            ):
                # PSUM budget (8 banks):
                #   A+g0 region: proj/tp ring 2 + sc 4 + av 2 = 8
                #   g1+C region: sc(+op) 4 + av 2 + bc 2 = 8
                # LIFO pool stack: psA (innermost) closes mid-kernel, psC
                # takes its banks
                psS_ctx = tc.tile_pool(name="psS", bufs=2, space="PSUM")
                psS = psS_ctx.__enter__()
                psAV_ctx = tc.tile_pool(name="psAV", bufs=1, space="PSUM")
                psAV = psAV_ctx.__enter__()
                psA_ctx = tc.tile_pool(name="psA", bufs=2, space="PSUM")
                psA = psA_ctx.__enter__()
                psC = None

                # ---------- phase A steps (one batch -> list of closures) ----
                def a_steps(b):
                    steps = []

                    def ones_cols():
                        # [v0|1|v1|1] slots: ones columns 64 and 129 feed the
                        # denominator row of the augmented AV matmuls
                        v3 = vA[b][:].rearrange("p (t c) -> p t c", c=130)
                        nc.vector.tensor_copy(v3[:, :, 64], onesm_sb[:])
                        nc.vector.tensor_copy(v3[:, :, 129], onesm_sb[:])
                    steps.append(ones_cols)

                    for ch in range(NQC):
                        xt = []

                        def dma_x(ch=ch, xt=xt):
                            i0 = b * T + ch * QC
                            for j in range(ND):
                                t = pA.tile([128, QC], DT, name=f"x{j}",
                                            tag=f"x{j}", bufs=2)
                                eng = nc.sync if j % 2 == 0 else nc.gpsimd
                                eng.dma_start(
                                    t[:],
                                    xt_d[j * 128:(j + 1) * 128, i0:i0 + QC])
                                xt.append(t)
                        steps.append(dma_x)

                        # q and k projections: 2 half-steps each
                        for w_sb, dstl in ((wq_sb, qT), (wk_sb, kT)):
                            pp = []

                            def proj1(w_sb=w_sb, xt=xt, pp=pp):
                                t = psA.tile([128, QC], f32, name="pp",
                                             tag="proj", bufs=2)
                                pp.append(t)
                                for j in range(4):
                                    nc.tensor.matmul(t[:], w_sb[j][:],
                                                     xt[j][:],
                                                     start=(j == 0),
                                                     stop=False)

                            def proj2(w_sb=w_sb, dstl=dstl, ch=ch, xt=xt,
                                      pp=pp):
                                t = pp[0]
                                for j in range(4, ND):
                                    nc.tensor.matmul(t[:], w_sb[j][:],
                                                     xt[j][:], start=False,
                                                     stop=(j == ND - 1))
                                sl = slice(ch * QC, (ch + 1) * QC)
                                nc.vector.tensor_copy(dstl[b][:, sl], t[:])
                            steps.append(proj1)
                            steps.append(proj2)

                        # v projection + transposes
                        vst = []

                        def projv1(xt=xt, vst=vst):
                            t = psA.tile([128, QC], f32, name="pp",
                                         tag="proj", bufs=2)
                            vst.append(t)
                            for j in range(4):
                                nc.tensor.matmul(t[:], wv_sb[j][:], xt[j][:],
                                                 start=(j == 0), stop=False)

                        def projv2(xt=xt, vst=vst):
                            t = vst[0]
                            for j in range(4, ND):
                                nc.tensor.matmul(t[:], wv_sb[j][:], xt[j][:],
                                                 start=False,
                                                 stop=(j == ND - 1))
                            s = pA.tile([128, QC], DT, name="vst", tag="vst",
                                        bufs=2)
                            nc.vector.tensor_copy(s[:], t[:])
                            vst.append(s)
                        steps.append(projv1)
                        steps.append(projv2)

                        def trans(ch=ch, vst=vst, lo=0):
                            s = vst[1]
                            for blk in range(lo, lo + 2):
                                tp = psA.tile([128, QC], DT, name="tp",
                                              tag="proj", bufs=2)
                                nc.tensor.transpose(
                                    tp[:, 0:128],
                                    s[:, blk * 128:(blk + 1) * 128], id_sb[:])
                                slot = (ch * (QC // 128) + blk)
                                # free-dim-split view (partition intact) —
                                # a partition-split view here would corrupt
                                dst = vA[b][:, slot * 130:slot * 130 + 130]\
                                    .rearrange("p (h c) -> p h c", c=65)[
                                        :, :, 0:64]
                                src = tp[:, 0:128].rearrange(
                                    "p (h e) -> p h e", e=64)
                                nc.vector.tensor_copy(dst, src)
                        steps.append(lambda trans=trans: trans(lo=0))
                        steps.append(lambda trans=trans: trans(lo=2))
                    return steps

                # ---------- filler ----------
                class Filler:
                    def __init__(self):
                        self.q = []

                    def add(self, steps):
                        self.q.extend(steps)

                    def step(self, n=1):
                        for _ in range(n):
                            if self.q:
                                self.q.pop(0)()

                    def drain(self):
                        while self.q:
                            self.q.pop(0)()

                filler = Filler()

                # ---------- attention chunk ----------
                def attn_chunk(b, qc):
                    nj = 4 * qc + 4
                    # ones-augmented AV: stationary [v_h|1] (M=65) — row 64
                    # accumulates the softmax denominator for free
                    av = [psAV.tile([65, QC], f32, name=f"av{h}",
                                    tag=f"av{h}", bufs=1)
                          for h in range(HPC)]
                    sc_t = {}
                    e_t = {}

                    def emit_scores(j):
                        jr = j - 4 * qc
                        off = max(jr, 0) * 128
                        w = QC - off
                        qsl = slice(qc * QC + off, (qc + 1) * QC)
                        sc = psS.tile([128, 2 * QC], f32, name="scb",
                                      tag="scb", bufs=2)
                        for h in range(HPC):
                            hp = slice(h * 64, (h + 1) * 64)
                            nc.tensor.matmul(
                                sc[:, h * QC:h * QC + w],
                                kT[b][hp, j * 128:(j + 1) * 128],
                                qT[b][hp, qsl], start=True, stop=True)
                        sc_t[j] = (sc, w)

                    def emit_exp_mask(j):
                        jr = j - 4 * qc
                        sc, w = sc_t.pop(j)
                        e = pB.tile([128, 2 * QC], DT, name="exb",
                                    tag="exb", bufs=3)
                        sc3 = sc[:].rearrange("p (two q) -> p two q",
                                              two=2)[:, :, 0:w]
                        e3 = e[:, 0:2 * w].rearrange("p (two q) -> p two q",
                                                     two=2)
                        nc.scalar.activation(e3, sc3, EXP,
                                             scale=1.0 / np.sqrt(HS))
                        if jr >= 0:
                            for h in range(HPC):
                                nc.vector.tensor_mul(
                                    e[:, h * w:h * w + 128],
                                    e[:, h * w:h * w + 128], triu_sb[:])
                        if DBG and b == 0 and qc == 0 and j == 0:
                            nc.sync.dma_start(dbg_e[:], e[:])
                        e_t[j] = (e, w)

                    def emit_av(j):
                        jr = j - 4 * qc
                        off = max(jr, 0) * 128
                        e, w = e_t.pop(j)
                        st, sp = (j == 0), (j == nj - 1)
                        for h in range(HPC):
                            lhs = vA[b][:, j * 130 + h * 65:
                                        j * 130 + h * 65 + 65]
                            nc.tensor.matmul(av[h][:, off:QC], lhs,
                                             e[:, h * w:(h + 1) * w],
                                             start=st, stop=sp)

                    # software pipeline: scores one j ahead of AV
                    emit_scores(0)
                    for j in range(nj):
                        emit_exp_mask(j)
                        if j + 1 < nj:
                            emit_scores(j + 1)
                        # filler lands between S(j+1) and AV(j) in the PE
                        # queue, covering the wait for exp(j)
                        filler.step(2)
                        emit_av(j)

                    # drain: raw ctx + denominator rows to a2a staging
                    tok0 = b * T + qc * QC
                    d = tok0 // SL
                    g = qc % NSPLIT
                    for h in range(HPC):
                        avs = pB.tile([65, QC], DT, name=f"avs{h}",
                                      tag=f"avs{h}", bufs=6)
                        nc.vector.tensor_copy(avs[:], av[h][:])
                        if DBG and b == 0 and qc == 0 and h == 0:
                            nc.sync.dma_start(dbg_avs[:], avs[:])
                        nc.gpsimd.dma_start(
                            a2a_in[g][d, h * 64:(h + 1) * 64, :],
                            avs[0:64, :])
                        nc.gpsimd.dma_start(
                            a2d_in[g][d, h:h + 1, :], avs[64:65, :])

                # ---------- phase C (one half) -> list of closures ----------
                def c_steps(g, last=False):
                    steps = []
                    cx = []
                    cxn = [None] * ND
                    rec_b = []

                    def load():
                        eng = nc.sync
                        if DBG and g == 0:
                            eng.dma_start(dbg_a2do[:], a2d_out[0][:])
                        # den_b rows h-major: r = h*8 + d.  (A partition-dim
                        # split rearrange as DMA dst writes byte-shifted
                        # garbage for h>0 — extract with two plain DMAs.)
                        den_b = pC.tile([2 * ND, HF], DT, name=f"den{g}",
                                        tag="den", bufs=2)
                        eng.dma_start(den_b[0:ND, :], a2d_out[g][:, 0, :])
                        eng.dma_start(den_b[ND:2 * ND, :],
                                      a2d_out[g][:, 1, :])
                        den_f = pC.tile([2 * ND, HF], f32, name=f"denf{g}",
                                        tag="denf", bufs=2)
                        nc.vector.tensor_copy(den_f[:], den_b[:])
                        rec_f = pC.tile([2 * ND, HF], f32, name=f"recf{g}",
                                        tag="recf", bufs=2)
                        nc.vector.reciprocal_approx_fast(rec_f[:], den_f[:])
                        rb = pC.tile([2 * ND, HF], DT, name=f"rec{g}",
                                     tag="rec", bufs=2)
                        nc.vector.tensor_copy(rb[:], rec_f[:])
                        rec_b.append(rb)
                        if DBG and g == 0:
                            nc.sync.dma_start(dbg_den[:], den_b[:])
                            nc.sync.dma_start(dbg_rec[:], rb[:])
                        # g=0's loads must avoid gpsimd (queued behind the
                        # group-1 collectives); g=1's go there to unload sync
                        cxe = nc.sync if g == 0 else nc.gpsimd
                        for j in range(ND):
                            t = pC.tile([128, HF], DT, name=f"cx{g}{j}",
                                        tag=f"cx{g}{j}", bufs=1)
                            cxe.dma_start(t[:], a2a_out[g][j])
                            cx.append(t)
                    steps.append(load)

                    def norm(j0):
                        for j in range(j0, j0 + 2):
                            bc = psC.tile([128, HF], f32, name="bc",
                                          tag="bc", bufs=2)
                            nc.tensor.matmul(bc[:],
                                             sel_sb[:, j * 128:(j + 1) * 128],
                                             rec_b[0][:], start=True,
                                             stop=True)
                            t = pC.tile([128, HF], DT, name=f"cxn{g}{j}",
                                        tag=f"cxn{g}{j}", bufs=1)
                            nc.vector.tensor_mul(t[:], cx[j][:], bc[:])
                            if DBG and g == 0 and j == 0:
                                nc.sync.dma_start(dbg_cxn[:], t[:])
                            cxn[j] = t
                    for j0 in range(0, ND, 2):
                        steps.append(lambda j0=j0: norm(j0))

                    def proj_m(m):
                        op = psS.tile([128, QC], f32, name="op", tag="scb",
                                      bufs=2)
                        for j in range(ND):
                            nc.tensor.matmul(
                                op[:], wp_sb[j][:, m * 128:(m + 1) * 128],
                                cxn[j][:], start=(j == 0), stop=(j == ND - 1))
                        os_ = pC.tile([128, QC], f32, name="os", tag="os",
                                      bufs=2)
                        nc.vector.tensor_scalar_add(os_[:], op[:], bp_sb[m][:])
                        nc.sync.dma_start(
                            out_d[m * 128:(m + 1) * 128,
                                  g * HF:(g + 1) * HF], os_[:])
                    for m in range(ND):
                        steps.append(lambda m=m: proj_m(m))
                    return steps

                # ---------- emission schedule ----------
                GROUPS = ([0, 2], [1, 3])

                with nc.named_scope("phA0"):
                    st0 = a_steps(0)
                    st0[0]()   # ones columns (DVE)
                    st0[1]()   # x chunk-0 DMAs
                    load_rest()
                    for st in st0[2:]:
                        st()
                    if DBG:
                        nc.sync.dma_start(dbg_q[:], qT[0][:])
                        nc.sync.dma_start(dbg_vA[:], vA[0][:])

                with nc.named_scope("phB0"):
                    for b in range(B):
                        if b + 1 < B:
                            filler.add(a_steps(b + 1))
                        for qc in GROUPS[0]:
                            attn_chunk(b, qc)
                        filler.drain()

                # close psA / open psC BEFORE the collectives: the pool
                # transition emits a cross-engine sync, which placed after
                # the collectives stalls every engine until the AllToAll
                # completes (~45us dead zone + HAM re-throttle)
                psA_ctx.__exit__(None, None, None)
                psC_ctx = tc.tile_pool(name="psC", bufs=2, space="PSUM")
                psC = psC_ctx.__enter__()

                load_wp()
                if DBG:
                    nc.sync.dma_start(dbg_a2di[:], a2d_in[0][:])
                nc.gpsimd.collective_compute(
                    "AllToAll", mybir.AluOpType.bypass,
                    replica_groups=[list(range(N_CORES))],
                    ins=[a2a_in[0].opt()], outs=[a2a_out[0].opt()])
                nc.gpsimd.collective_compute(
                    "AllToAll", mybir.AluOpType.bypass,
                    replica_groups=[list(range(N_CORES))],
                    ins=[a2d_in[0].opt()], outs=[a2d_out[0].opt()])

                with nc.named_scope("phB1"):
                    for b in range(B):
                        for qc in GROUPS[1]:
                            attn_chunk(b, qc)

                # group-1 collectives BEFORE phC0: phC0's compute (which only
                # needs group 0's data) overlaps their transport
                nc.gpsimd.collective_compute(
                    "AllToAll", mybir.AluOpType.bypass,
                    replica_groups=[list(range(N_CORES))],
                    ins=[a2d_in[1].opt()], outs=[a2d_out[1].opt()])
                nc.gpsimd.collective_compute(
                    "AllToAll", mybir.AluOpType.bypass,
                    replica_groups=[list(range(N_CORES))],
                    ins=[a2a_in[1].opt()], outs=[a2a_out[1].opt()])

                with nc.named_scope("phC0"):
                    for st in c_steps(0):
                        st()
                with nc.named_scope("phC1"):
                    for st in c_steps(1, last=True):
                        st()

                for c in (psC_ctx, psAV_ctx, psS_ctx):
                    c.__exit__(None, None, None)

    nc.compile()
    return nc


def prep_inputs(x, Wq, Wk, Wv, Wp, bp, T, dt_name=DT_NAME):
    """Host-side sharding/layout prep. Returns in_maps for the 8 cores."""
    DT = {"f32r": f32r, "bf16": bf16, "f32": f32}[dt_name]
    ndt = _np_dt(DT)
    BT = B * T
    NTB = T // KT

    x = np.asarray(x, np.float32)
    Wq = np.asarray(Wq, np.float32)
    Wk = np.asarray(Wk, np.float32)
    Wv = np.asarray(Wv, np.float32)
    Wp = np.asarray(Wp, np.float32)
    bp = np.asarray(bp, np.float32)

    xt = np.ascontiguousarray(x.reshape(BT, D).T).astype(ndt)
    wp = np.ascontiguousarray(Wp.T).astype(ndt)
    bpc = np.ascontiguousarray(bp.reshape(D, 1))
    ident = np.eye(128, dtype=np.float32).astype(ndt)
    triu = np.triu(np.ones((128, 128), np.float32)).astype(ndt)
    onesm = np.ones((128, NTB), np.float32).astype(ndt)
    # selT[r, j*128 + p] = 1 iff r == 8*(p//64) + j  (recip rows are h-major:
    # r = h*8 + slot; cx_j partition p belongs to head h = p//64 of slot j)
    sel = np.zeros((2 * ND, ND * 128), np.float32)
    for j in range(ND):
        for p in range(128):
            sel[8 * (p // 64) + j, j * 128 + p] = 1.0
    sel = sel.astype(ndt)

    def wslice(W, c):
        # [H, D, HS] heads 2c,2c+1 -> [D, 128] as [d, (h_local, e)]
        return np.ascontiguousarray(
            W[2 * c:2 * c + 2].transpose(1, 0, 2).reshape(D, 2 * HS)).astype(ndt)

    in_maps = []
    for c in range(N_CORES):
        in_maps.append({
            "xt": xt, "wq": wslice(Wq, c), "wk": wslice(Wk, c),
            "wv": wslice(Wv, c), "wp": wp, "bp": bpc, "ident": ident,
            "triu": triu, "onesm": onesm, "selT": sel,
        })
    return in_maps


_NC_CACHE = {}


def kernel(x, Wq, Wk, Wv, Wp, bp):
    T = np.asarray(x).shape[1]
    key = (T, DT_NAME)
    if key not in _NC_CACHE:
        _NC_CACHE[key] = build_nc(T, DT_NAME)
    nc = _NC_CACHE[key]
    in_maps = prep_inputs(x, Wq, Wk, Wv, Wp, bp, T, DT_NAME)
    res = run_bass_kernel_spmd(nc, in_maps, list(range(N_CORES)))
    out = np.concatenate([res.results[c]["outT"].T for c in range(N_CORES)],
                         axis=0)
    return np.ascontiguousarray(out.reshape(B, T, D).astype(np.float32))


# revision 68
# speedup vs baseline: 1.2117x; 1.0106x over previous
"""Multi-head causal attention (B=4, T=2048, D=1024, H=16, HS=64) on 8 TRN2 cores.

Sharding: tensor-parallel over heads (2 heads/core) for QKV+attention, then an
AllToAll redistributes per-head context to token-parallel layout for the output
projection (each core projects 1024 tokens with the full Wp).

v2 structure (vs v1): the kernel is ACT(exp)/PE-paced, so everything else is
arranged to keep ScalarE doing exp only and TensorE dense:
  - all PSUM->SBUF evacuations run on DVE (ScalarE does exp exclusively)
  - softmax normalization is deferred past the AllToAll: phase B ships raw
    context rows plus per-(head,token) denominators (computed by K=128
    ones-column matmuls, col-packed with the two heads' AV matmuls); phase C
    does one fp32 reciprocal_approx_fast on DVE + one K=16 broadcast matmul
    per d-tile. No ACT reciprocal => no ACT table switching (the exp table
    set loads once). Context and denominators travel in separate AllToAlls
    (shards beyond 128KiB arrive corrupted; ctx is exactly 128KiB/shard).
  - emission interleaves phase A(b+1) matmuls between attention j-steps of
    batch b (and phase C0 between group-1 chunks) so the PE queue always has
    independent work behind score->exp->AV dependency stalls; scores are
    software-pipelined one j ahead of AV.
  - x-tile loads alternate between the sync and gpsimd DMA queues; a2a staging
    uses gpsimd; phase-C loads/stores use sync.

Compute dtype bf16 with fp32 PSUM accumulation everywhere.
"""
import os
import numpy as np

import concourse.bass as bass
import concourse.tile as tile
from concourse import bacc, mybir
from concourse.bass_utils import run_bass_kernel_spmd

f32 = mybir.dt.float32
f32r = mybir.dt.float32r
bf16 = mybir.dt.bfloat16

B, D, H, HS = 4, 1024, 16, 64
N_CORES = 8
HPC = H // N_CORES          # heads per core
QC = 512                    # q-chunk width
KT = 128                    # k-tile width
ND = D // 128               # din tiles

DT_NAME = "bf16"            # "f32r" | "bf16" | "f32"


def _np_dt(dt):
    import ml_dtypes
    return {f32: np.float32, f32r: np.float32, bf16: ml_dtypes.bfloat16}[dt]


def build_nc(T=2048, dt_name=DT_NAME):
    DT = {"f32r": f32r, "bf16": bf16, "f32": f32}[dt_name]
    BT = B * T
    SL = BT // N_CORES              # tokens per core in phase C
    NQC = T // QC                   # q-chunks per batch
    NTB = T // KT                   # k/t-tiles per batch
    NSPLIT = 4                      # a2a groups (g = 2*(qc%2) + col-half)
    HF = SL // NSPLIT

    nc = bacc.Bacc("TRN2", target_bir_lowering=False, debug=False,
                   num_devices=N_CORES)

    xt_d = nc.dram_tensor("xt", [D, BT], DT, kind="ExternalInput").ap()
    wq_d = nc.dram_tensor("wq", [D, 128], DT, kind="ExternalInput").ap()
    wk_d = nc.dram_tensor("wk", [D, 128], DT, kind="ExternalInput").ap()
    wv_d = nc.dram_tensor("wv", [D, 128], DT, kind="ExternalInput").ap()
    wp_d = nc.dram_tensor("wp", [D, D], DT, kind="ExternalInput").ap()
    bp_d = nc.dram_tensor("bp", [D, 1], f32, kind="ExternalInput").ap()
    id_d = nc.dram_tensor("ident", [128, 128], DT, kind="ExternalInput").ap()
    tril_d = nc.dram_tensor("triu", [128, 128], DT, kind="ExternalInput").ap()
    onesm_d = nc.dram_tensor("onesm", [128, NTB], DT, kind="ExternalInput").ap()
    sel_d = nc.dram_tensor("selT", [2 * ND, ND * 128], DT,
                           kind="ExternalInput").ap()
    out_d = nc.dram_tensor("outT", [D, SL], f32, kind="ExternalOutput").ap()

    DBG = bool(os.environ.get("KERN_DBG"))
    if DBG:
        dbg_q = nc.dram_tensor("dbg_q", [128, T], DT,
                               kind="ExternalOutput").ap()
        dbg_vA = nc.dram_tensor("dbg_vA", [128, (T // KT) * 128], DT,
                                kind="ExternalOutput").ap()
        dbg_avs = nc.dram_tensor("dbg_avs", [65, QC], DT,
                                 kind="ExternalOutput").ap()
        dbg_e = nc.dram_tensor("dbg_e", [128, 2 * QC], DT,
                               kind="ExternalOutput").ap()
        dbg_den = nc.dram_tensor("dbg_den", [2 * ND, HF], DT,
                                 kind="ExternalOutput").ap()
        dbg_rec = nc.dram_tensor("dbg_rec", [2 * ND, HF], DT,
                                 kind="ExternalOutput").ap()
        dbg_cxn = nc.dram_tensor("dbg_cxn", [128, HF], DT,
                                 kind="ExternalOutput").ap()
        dbg_dns = nc.dram_tensor("dbg_dns", [33, QC], DT,
                                 kind="ExternalOutput").ap()
        dbg_a2di = nc.dram_tensor("dbg_a2di", [N_CORES, 4, HF], DT,
                                  kind="ExternalOutput").ap()
        dbg_a2do = nc.dram_tensor("dbg_a2do", [N_CORES, 4, HF], DT,
                                  kind="ExternalOutput").ap()

    EXP = mybir.ActivationFunctionType.Exp

    with tile.TileContext(nc) as tc:
        with (
            tc.tile_pool(name="wts", bufs=1) as wts,
            tc.tile_pool(name="acts", bufs=1) as acts,
            tc.tile_pool(name="dram", bufs=1, space="DRAM") as dram,
        ):
            # ---- persistent tiles.  Each DMA costs ~600ns of queue issue
            # time, so only Wq (needed by the very first matmul) loads before
            # the first x chunk; the rest loads right after (load_rest) ----
            wq_sb, wk_sb, wv_sb = [], [], []
            for j in range(ND):
                for lst, nm in ((wq_sb, "wq"), (wk_sb, "wk"), (wv_sb, "wv")):
                    lst.append(wts.tile([128, 128], DT, name=f"{nm}{j}",
                                        tag=f"{nm}{j}"))
            id_sb = wts.tile([128, 128], DT, name="id", tag="id")
            onesm_sb = wts.tile([128, NTB], DT, name="onesm", tag="onesm")
            triu_sb = wts.tile([128, 128], DT, name="triu", tag="triu")
            sel_sb = wts.tile([2 * ND, ND * 128], DT, name="selT", tag="selT")

            for j in range(ND):
                eng = nc.sync if j % 2 == 0 else nc.gpsimd
                eng.dma_start(wq_sb[j][:], wq_d[j * 128:(j + 1) * 128, :])

            def load_rest():
                for lst, dd in ((wk_sb, wk_d), (wv_sb, wv_d)):
                    for j in range(ND):
                        eng = nc.sync if j % 2 == 0 else nc.gpsimd
                        eng.dma_start(lst[j][:],
                                      dd[j * 128:(j + 1) * 128, :])
                nc.gpsimd.dma_start(id_sb[:], id_d[:])
                nc.gpsimd.dma_start(onesm_sb[:], onesm_d[:])
                nc.gpsimd.dma_start(triu_sb[:], tril_d[:])
                nc.gpsimd.dma_start(sel_sb[:], sel_d[:])

            # a2a buffers.  Shard payloads beyond 128KiB arrive corrupted
            # (measured: rows >=128 of a [8,130,512]bf16 and [8,132,512]bf16
            # buffer are garbage after the collective), so context rows
            # ([8,128,HF] = exactly 128KiB/shard, the known-good shape) and
            # denominator rows ([8,4,HF], one full 4KiB CCE chunk) travel in
            # separate AllToAlls.
            a2a_in = [dram.tile([N_CORES, 128, HF], DT, name=f"a2ai{g}")
                      for g in range(NSPLIT)]
            a2a_out = [dram.tile([N_CORES, 128, HF], DT, name=f"a2ao{g}")
                       for g in range(NSPLIT)]
            a2d_in = [dram.tile([N_CORES, 8, HF], DT, name=f"a2di{g}")
                      for g in range(NSPLIT)]
            a2d_out = [dram.tile([N_CORES, 8, HF], DT, name=f"a2do{g}")
                       for g in range(NSPLIT)]

            # per-batch activation tensors
            qT, kT, vA = [], [], []
            for b in range(B):
                qT.append(acts.tile([128, T], DT, name=f"qT{b}", tag=f"qT{b}"))
                kT.append(acts.tile([128, T], DT, name=f"kT{b}", tag=f"kT{b}"))
                vA.append(acts.tile([128, NTB * 130], DT, name=f"vA{b}",
                                    tag=f"vA{b}"))

            # wp/bp aren't needed until phase C — loaded late (just before
            # the first AllToAll) so phase A owns the DMA queues at startup
            wp_sb = []
            bp_sb = []

            def load_wp():
                for j in range(ND):
                    t = wts.tile([128, D], DT, name=f"wp{j}", tag=f"wp{j}")
                    nc.sync.dma_start(t[:], wp_d[j * 128:(j + 1) * 128, :])
                    wp_sb.append(t)
                for m in range(ND):
                    t = wts.tile([128, 1], f32, name=f"bp{m}", tag=f"bp{m}")
                    nc.sync.dma_start(t[:], bp_d[m * 128:(m + 1) * 128, :])
                    bp_sb.append(t)

            with (
                tc.tile_pool(name="pA", bufs=2) as pA,
                tc.tile_pool(name="pB", bufs=3) as pB,
                tc.tile_pool(name="pC", bufs=1) as pC,
            ):
                # PSUM budget (8 banks):
                #   A+g0 region: proj/tp ring 2 + sc 4 + av 2 = 8
                #   g1+C region: sc(+op) 4 + av 2 + bc 2 = 8
                # PSUM: proj/tp/bc share one 2-slot ring (phase C's bc tiles
                # reuse phase A's banks via the same tag — no pool close, no
                # cross-engine barrier); sc/op 4 banks; av 2 banks = 8 total
                psS_ctx = tc.tile_pool(name="psS", bufs=2, space="PSUM")
                psS = psS_ctx.__enter__()
                psAV_ctx = tc.tile_pool(name="psAV", bufs=1, space="PSUM")
                psAV = psAV_ctx.__enter__()
                psA_ctx = tc.tile_pool(name="psA", bufs=2, space="PSUM")
                psA = psA_ctx.__enter__()

                # ---------- phase A steps (one batch -> list of closures) ----
                def a_steps(b):
                    steps = []

                    def ones_cols():
                        # [v0|1|v1|1] slots: ones columns 64 and 129 feed the
                        # denominator row of the augmented AV matmuls
                        v3 = vA[b][:].rearrange("p (t c) -> p t c", c=130)
                        nc.vector.tensor_copy(v3[:, :, 64], onesm_sb[:])
                        nc.vector.tensor_copy(v3[:, :, 129], onesm_sb[:])
                    steps.append(ones_cols)

                    for ch in range(NQC):
                        xt = []

                        def dma_x(ch=ch, xt=xt):
                            i0 = b * T + ch * QC
                            for j in range(ND):
                                t = pA.tile([128, QC], DT, name=f"x{j}",
                                            tag=f"x{j}", bufs=2)
                                eng = nc.sync if j % 2 == 0 else nc.gpsimd
                                eng.dma_start(
                                    t[:],
                                    xt_d[j * 128:(j + 1) * 128, i0:i0 + QC])
                                xt.append(t)
                        steps.append(dma_x)

                        # q and k projections: 2 half-steps each
                        for w_sb, dstl in ((wq_sb, qT), (wk_sb, kT)):
                            pp = []

                            def proj1(w_sb=w_sb, xt=xt, pp=pp):
                                t = psA.tile([128, QC], f32, name="pp",
                                             tag="proj", bufs=2)
                                pp.append(t)
                                for j in range(4):
                                    nc.tensor.matmul(t[:], w_sb[j][:],
                                                     xt[j][:],
                                                     start=(j == 0),
                                                     stop=False)

                            def proj2(w_sb=w_sb, dstl=dstl, ch=ch, xt=xt,
                                      pp=pp):
                                t = pp[0]
                                for j in range(4, ND):
                                    nc.tensor.matmul(t[:], w_sb[j][:],
                                                     xt[j][:], start=False,
                                                     stop=(j == ND - 1))
                                sl = slice(ch * QC, (ch + 1) * QC)
                                nc.vector.tensor_copy(dstl[b][:, sl], t[:])
                            steps.append(proj1)
                            steps.append(proj2)

                        # v projection + transposes
                        vst = []

                        def projv1(xt=xt, vst=vst):
                            t = psA.tile([128, QC], f32, name="pp",
                                         tag="proj", bufs=2)
                            vst.append(t)
                            for j in range(4):
                                nc.tensor.matmul(t[:], wv_sb[j][:], xt[j][:],
                                                 start=(j == 0), stop=False)

                        def projv2(xt=xt, vst=vst):
                            t = vst[0]
                            for j in range(4, ND):
                                nc.tensor.matmul(t[:], wv_sb[j][:], xt[j][:],
                                                 start=False,
                                                 stop=(j == ND - 1))
                            s = pA.tile([128, QC], DT, name="vst", tag="vst",
                                        bufs=2)
                            nc.vector.tensor_copy(s[:], t[:])
                            vst.append(s)
                        steps.append(projv1)
                        steps.append(projv2)

                        def trans(ch=ch, vst=vst, lo=0):
                            s = vst[1]
                            for blk in range(lo, lo + 2):
                                tp = psA.tile([128, QC], DT, name="tp",
                                              tag="proj", bufs=2)
                                nc.tensor.transpose(
                                    tp[:, 0:128],
                                    s[:, blk * 128:(blk + 1) * 128], id_sb[:])
                                slot = (ch * (QC // 128) + blk)
                                # free-dim-split view (partition intact) —
                                # a partition-split view here would corrupt
                                dst = vA[b][:, slot * 130:slot * 130 + 130]\
                                    .rearrange("p (h c) -> p h c", c=65)[
                                        :, :, 0:64]
                                src = tp[:, 0:128].rearrange(
                                    "p (h e) -> p h e", e=64)
                                nc.vector.tensor_copy(dst, src)
                        steps.append(lambda trans=trans: trans(lo=0))
                        steps.append(lambda trans=trans: trans(lo=2))
                    return steps

                # ---------- filler ----------
                class Filler:
                    def __init__(self):
                        self.q = []

                    def add(self, steps):
                        self.q.extend(steps)

                    def step(self, n=1):
                        for _ in range(n):
                            if self.q:
                                self.q.pop(0)()

                    def drain(self):
                        while self.q:
                            self.q.pop(0)()

                filler = Filler()

                # ---------- attention chunk ----------
                def attn_chunk(b, qc):
                    nj = 4 * qc + 4
                    # ones-augmented AV: stationary [v_h|1] (M=65) — row 64
                    # accumulates the softmax denominator for free
                    av = [psAV.tile([65, QC], f32, name=f"av{h}",
                                    tag=f"av{h}", bufs=1)
                          for h in range(HPC)]
                    sc_t = {}
                    e_t = {}

                    def emit_scores(j):
                        jr = j - 4 * qc
                        off = max(jr, 0) * 128
                        w = QC - off
                        qsl = slice(qc * QC + off, (qc + 1) * QC)
                        sc = psS.tile([128, 2 * QC], f32, name="scb",
                                      tag="scb", bufs=2)
                        for h in range(HPC):
                            hp = slice(h * 64, (h + 1) * 64)
                            nc.tensor.matmul(
                                sc[:, h * QC:h * QC + w],
                                kT[b][hp, j * 128:(j + 1) * 128],
                                qT[b][hp, qsl], start=True, stop=True)
                        sc_t[j] = (sc, w)

                    def emit_exp_mask(j):
                        jr = j - 4 * qc
                        sc, w = sc_t.pop(j)
                        e = pB.tile([128, 2 * QC], DT, name="exb",
                                    tag="exb", bufs=3)
                        sc3 = sc[:].rearrange("p (two q) -> p two q",
                                              two=2)[:, :, 0:w]
                        e3 = e[:, 0:2 * w].rearrange("p (two q) -> p two q",
                                                     two=2)
                        nc.scalar.activation(e3, sc3, EXP,
                                             scale=1.0 / np.sqrt(HS))
                        if jr >= 0:
                            for h in range(HPC):
                                nc.vector.tensor_mul(
                                    e[:, h * w:h * w + 128],
                                    e[:, h * w:h * w + 128], triu_sb[:])
                        if DBG and b == 0 and qc == 0 and j == 0:
                            nc.sync.dma_start(dbg_e[:], e[:])
                        e_t[j] = (e, w)

                    def emit_av(j):
                        jr = j - 4 * qc
                        off = max(jr, 0) * 128
                        e, w = e_t.pop(j)
                        st, sp = (j == 0), (j == nj - 1)
                        for h in range(HPC):
                            lhs = vA[b][:, j * 130 + h * 65:
                                        j * 130 + h * 65 + 65]
                            nc.tensor.matmul(av[h][:, off:QC], lhs,
                                             e[:, h * w:(h + 1) * w],
                                             start=st, stop=sp)

                    # software pipeline: scores one j ahead of AV
                    emit_scores(0)
                    for j in range(nj):
                        emit_exp_mask(j)
                        if j + 1 < nj:
                            emit_scores(j + 1)
                        # filler lands between S(j+1) and AV(j) in the PE
                        # queue, covering the wait for exp(j)
                        filler.step(2)
                        emit_av(j)

                    # drain: raw ctx + denominator rows to a2a staging
                    tok0 = b * T + qc * QC
                    d = tok0 // SL
                    for h in range(HPC):
                        avs = pB.tile([65, QC], DT, name=f"avs{h}",
                                      tag=f"avs{h}", bufs=6)
                        nc.vector.tensor_copy(avs[:], av[h][:])
                        for s in range(2):
                            g = 2 * (qc % 2) + s
                            cs = slice(s * HF, (s + 1) * HF)
                            nc.gpsimd.dma_start(
                                a2a_in[g][d, h * 64:(h + 1) * 64, :],
                                avs[0:64, cs])
                            nc.gpsimd.dma_start(
                                a2d_in[g][d, h:h + 1, :], avs[64:65, cs])

                # ---------- phase C (one half) -> list of closures ----------
                def c_steps(g, last=False):
                    steps = []
                    cx = []
                    cxn = [None] * ND
                    rec_b = []

                    def load():
                        eng = nc.sync
                        if DBG and g == 0:
                            eng.dma_start(dbg_a2do[:], a2d_out[0][:])
                        # den_b rows h-major: r = h*8 + d.  (A partition-dim
                        # split rearrange as DMA dst writes byte-shifted
                        # garbage for h>0 — extract with two plain DMAs.)
                        den_b = pC.tile([2 * ND, HF], DT, name=f"den{g}",
                                        tag="den", bufs=2)
                        eng.dma_start(den_b[0:ND, :], a2d_out[g][:, 0, :])
                        eng.dma_start(den_b[ND:2 * ND, :],
                                      a2d_out[g][:, 1, :])
                        den_f = pC.tile([2 * ND, HF], f32, name=f"denf{g}",
                                        tag="denf", bufs=2)
                        nc.vector.tensor_copy(den_f[:], den_b[:])
                        rec_f = pC.tile([2 * ND, HF], f32, name=f"recf{g}",
                                        tag="recf", bufs=2)
                        nc.vector.reciprocal_approx_fast(rec_f[:], den_f[:])
                        rb = pC.tile([2 * ND, HF], DT, name=f"rec{g}",
                                     tag="rec", bufs=2)
                        nc.vector.tensor_copy(rb[:], rec_f[:])
                        rec_b.append(rb)
                        if DBG and g == 0:
                            nc.sync.dma_start(dbg_den[:], den_b[:])
                            nc.sync.dma_start(dbg_rec[:], rb[:])
                        # g=0's loads must avoid gpsimd (queued behind the
                        # group-1 collectives); g=1's go there to unload sync
                        cxe = nc.sync if g == 0 else nc.gpsimd
                        for j in range(ND):
                            t = pC.tile([128, HF], DT, name=f"cx{g}{j}",
                                        tag=f"cx{g}{j}", bufs=1)
                            cxe.dma_start(t[:], a2a_out[g][j])
                            cx.append(t)
                    steps.append(load)

                    def norm(j0):
                        for j in range(j0, j0 + 2):
                            bc = psA.tile([128, HF], f32, name="bc",
                                          tag="proj", bufs=2)
                            nc.tensor.matmul(bc[:],
                                             sel_sb[:, j * 128:(j + 1) * 128],
                                             rec_b[0][:], start=True,
                                             stop=True)
                            t = pC.tile([128, HF], DT, name=f"cxn{g}{j}",
                                        tag=f"cxn{g}{j}", bufs=1)
                            nc.vector.tensor_mul(t[:], cx[j][:], bc[:])
                            if DBG and g == 0 and j == 0:
                                nc.sync.dma_start(dbg_cxn[:], t[:])
                            cxn[j] = t
                    for j0 in range(0, ND, 2):
                        steps.append(lambda j0=j0: norm(j0))

                    def proj_m(m):
                        op = psS.tile([128, HF], f32, name="op", tag="scb",
                                      bufs=2)
                        for j in range(ND):
                            nc.tensor.matmul(
                                op[:], wp_sb[j][:, m * 128:(m + 1) * 128],
                                cxn[j][:], start=(j == 0), stop=(j == ND - 1))
                        os_ = pC.tile([128, HF], f32, name="os", tag="os",
                                      bufs=2)
                        nc.vector.tensor_scalar_add(os_[:], op[:], bp_sb[m][:])
                        nc.sync.dma_start(
                            out_d[m * 128:(m + 1) * 128,
                                  g * HF:(g + 1) * HF], os_[:])
                    for m in range(ND):
                        steps.append(lambda m=m: proj_m(m))
                    return steps

                # ---------- emission schedule ----------
                GROUPS = ([0, 2], [1, 3])

                with nc.named_scope("phA0"):
                    st0 = a_steps(0)
                    st0[1]()   # x chunk-0 DMAs first
                    load_rest()
                    st0[0]()   # ones columns (needs onesm loaded first!)
                    for st in st0[2:]:
                        st()
                    if DBG:
                        nc.sync.dma_start(dbg_q[:], qT[0][:])
                        nc.sync.dma_start(dbg_vA[:], vA[0][:])

                with nc.named_scope("phB0"):
                    for b in range(B):
                        if b + 1 < B:
                            filler.add(a_steps(b + 1))
                        for qc in GROUPS[0]:
                            attn_chunk(b, qc)
                        filler.drain()

                load_wp()
                for g in (0, 1):
                    nc.gpsimd.collective_compute(
                        "AllToAll", mybir.AluOpType.bypass,
                        replica_groups=[list(range(N_CORES))],
                        ins=[a2d_in[g].opt()], outs=[a2d_out[g].opt()])
                    nc.gpsimd.collective_compute(
                        "AllToAll", mybir.AluOpType.bypass,
                        replica_groups=[list(range(N_CORES))],
                        ins=[a2a_in[g].opt()], outs=[a2a_out[g].opt()])

                with nc.named_scope("phB1"):
                    for b in range(B):
                        for qc in GROUPS[1]:
                            attn_chunk(b, qc)

                # groups 2/3 collectives BEFORE phC: phC0+phC1 compute
                # (needing only groups 0/1) overlaps their transport
                for g in (2, 3):
                    nc.gpsimd.collective_compute(
                        "AllToAll", mybir.AluOpType.bypass,
                        replica_groups=[list(range(N_CORES))],
                        ins=[a2d_in[g].opt()], outs=[a2d_out[g].opt()])
                    nc.gpsimd.collective_compute(
                        "AllToAll", mybir.AluOpType.bypass,
                        replica_groups=[list(range(N_CORES))],
                        ins=[a2a_in[g].opt()], outs=[a2a_out[g].opt()])

                for g in range(NSPLIT):
                    with nc.named_scope(f"phC{g}"):
                        for st in c_steps(g, last=(g == NSPLIT - 1)):
                            st()

                for c in (psA_ctx, psAV_ctx, psS_ctx):
                    c.__exit__(None, None, None)

    nc.compile()
    return nc


def prep_inputs(x, Wq, Wk, Wv, Wp, bp, T, dt_name=DT_NAME):
    """Host-side sharding/layout prep. Returns in_maps for the 8 cores."""
    DT = {"f32r": f32r, "bf16": bf16, "f32": f32}[dt_name]
    ndt = _np_dt(DT)
    BT = B * T
    NTB = T // KT

    x = np.asarray(x, np.float32)
    Wq = np.asarray(Wq, np.float32)
    Wk = np.asarray(Wk, np.float32)
    Wv = np.asarray(Wv, np.float32)
    Wp = np.asarray(Wp, np.float32)
    bp = np.asarray(bp, np.float32)

    xt = np.ascontiguousarray(x.reshape(BT, D).T).astype(ndt)
    wp = np.ascontiguousarray(Wp.T).astype(ndt)
    bpc = np.ascontiguousarray(bp.reshape(D, 1))
    ident = np.eye(128, dtype=np.float32).astype(ndt)
    triu = np.triu(np.ones((128, 128), np.float32)).astype(ndt)
    onesm = np.ones((128, NTB), np.float32).astype(ndt)
    # selT[r, j*128 + p] = 1 iff r == 8*(p//64) + j  (recip rows are h-major:
    # r = h*8 + slot; cx_j partition p belongs to head h = p//64 of slot j)
    sel = np.zeros((2 * ND, ND * 128), np.float32)
    for j in range(ND):
        for p in range(128):
            sel[8 * (p // 64) + j, j * 128 + p] = 1.0
    sel = sel.astype(ndt)

    def wslice(W, c):
        # [H, D, HS] heads 2c,2c+1 -> [D, 128] as [d, (h_local, e)]
        return np.ascontiguousarray(
            W[2 * c:2 * c + 2].transpose(1, 0, 2).reshape(D, 2 * HS)).astype(ndt)

    in_maps = []
    for c in range(N_CORES):
        in_maps.append({
            "xt": xt, "wq": wslice(Wq, c), "wk": wslice(Wk, c),
            "wv": wslice(Wv, c), "wp": wp, "bp": bpc, "ident": ident,
            "triu": triu, "onesm": onesm, "selT": sel,
        })
    return in_maps


_NC_CACHE = {}


def kernel(x, Wq, Wk, Wv, Wp, bp):
    T = np.asarray(x).shape[1]
    key = (T, DT_NAME)
    if key not in _NC_CACHE:
        _NC_CACHE[key] = build_nc(T, DT_NAME)
    nc = _NC_CACHE[key]
    in_maps = prep_inputs(x, Wq, Wk, Wv, Wp, bp, T, DT_NAME)
    res = run_bass_kernel_spmd(nc, in_maps, list(range(N_CORES)))
    out = np.concatenate([res.results[c]["outT"].T for c in range(N_CORES)],
                         axis=0)
    return np.ascontiguousarray(out.reshape(B, T, D).astype(np.float32))
